# revision 14
# baseline (speedup 1.0000x reference)
"""Trainium2 Bass kernel for BertSelfAttention with C_prior multiply.

Reference (per batch b):
  q/k/v = x @ W{q,k,v}.T + b{q,k,v}            -> [S, D], H=16 heads of W=64
  scores = q k^T / sqrt(W); mask; softmax over k
  attn = softmax(scores) * C_prior[b]
  out = attn @ v                               -> [B, S, D]

Shapes: B=2, S=2048, D=1024, H=16, W=64.
Sharding: 8 cores; core c owns batch b=c//4 and 4 heads (hg=c%4).

Measured HW model driving the design: PE matmul time ~ max(PSUM f32
writes / 265G/s, rhs cols / 3.9G/s); Act = 128 lanes @1.2GHz; DVE gets
2x on 16-bit SBUF ops; Pool engine (nc.gpsimd) is a second 1.2GHz
vector engine.

Design vs the naive version:
  - Projections in fp8e4m3 DoubleRow: contraction 256 per step -> 4
    PSUM accumulation passes instead of 8 (proj PE time halves).
    Weights are pre-scaled x8 on host (so they sit in e4m3 normal
    range); compensated via the exp scale (q,k) and the broadcast
    constant (v).
  - scores^T [k,q] per (strip, head) land in a [128,1024] PSUM tile
    (2 banks); ONE exp per (strip, head) -> halves Act instruction count.
  - e is stored fp16. The softmax denominator is computed by a
    DoubleRow fp8e5m2 matmul whose rhs is the strided high-byte view
    of the fp16 e tile (fp16 high byte == e5m2 truncation). exp is
    biased by ln(1.0625) (half an m2 ulp) so truncation becomes
    round-to-nearest; the 1.0625 cancels between numerator and
    denominator. Thin [1,512] outputs: denominator streams 2 strips
    per call and writes almost nothing.
  - 1/denom via thin reciprocal (DVE) then broadcast up to 128
    partitions with a tiny f32r matmul whose host constant also holds
    the 1/8 fp8-V compensation.
  - mask folded into the per-partition exp bias; attn*C multiplies
    split between DVE and Pool; projection bias-adds (PSUM->SBUF
    moves) on Pool.
"""

import math
import os

import numpy as np
import ml_dtypes

B, S, D, H, W = 2, 2048, 1024, 16, 64
NCORES = 8
HEADS_PER_CORE = 4
P = 128
QH = S // 2  # q processed in two halves of 1024
NK = S // P  # 16 k-strips
BOFF = 6  # pass B trails pass A by this many strips

USE_FP8_PROJ = False
ROUND_BIAS = 0.0  # no pre-scale: truncation bias is scale-invariant
E5_TRUNC_RATIO = 0.91578  # E[e5m2_trunc(x)/x] for exp(N(0,1)) mantissas
W8 = 8.0  # host pre-scale on all projection weights (fp8 range)

_prog_cache = {}


def _build_program():
    import concourse.mybir as mybir
    import concourse.tile as tile
    from concourse import bacc

    dt = mybir.dt
    f32, bf16, f16 = dt.float32, dt.bfloat16, dt.float16
    f8e4, f8e5, f32r = dt.float8e4, dt.float8e5, dt.float32r
    Alu = mybir.AluOpType
    Act = mybir.ActivationFunctionType
    DR = mybir.MatmulPerfMode.DoubleRow

    nc = bacc.Bacc("TRN2", target_bir_lowering=False)

    if USE_FP8_PROJ:
        # x/W laid out for DoubleRow: contraction d = t*256 + i*128 + p
        xq_d = nc.declare_dram_parameter("xq", [P, 4, 2, S], f8e4, isOutput=False)
        wqk_d = nc.declare_dram_parameter("wqk", [P, 4, 2, 512], f8e4, isOutput=False)
        wv_d = nc.declare_dram_parameter("wv", [P, 4, 2, 256], f8e4, isOutput=False)
    else:
        xq_d = nc.declare_dram_parameter("xq", [P, 8, S], bf16, isOutput=False)
        wqk_d = nc.declare_dram_parameter("wqk", [P, 8, 512], bf16, isOutput=False)
        wv_d = nc.declare_dram_parameter("wv", [P, 8, 256], bf16, isOutput=False)
    bqk_d = nc.declare_dram_parameter("bqk", [P, 4], f32, isOutput=False)
    bvr_d = nc.declare_dram_parameter("bvr", [P, 256], f32, isOutput=False)
    ct_d = nc.declare_dram_parameter("ct", [S, S], f16, isOutput=False)
    mkb_d = nc.declare_dram_parameter("mkb", [P, NK], f32, isOutput=False)
    ones_d = nc.declare_dram_parameter("ones3", [P, 2, 64], f8e5, isOutput=False)
    onesb_d = nc.declare_dram_parameter("onesb", [P, 64], f16, isOutput=False)
    dcmp_d = nc.declare_dram_parameter("dcmp", [P, 1], f32, isOutput=False)
    out_d = nc.declare_dram_parameter("out", [256, S], f32, isOutput=True)

    EXPSCALE = 0.125 / (W8 * W8) if USE_FP8_PROJ else 0.125

    with tile.TileContext(nc) as tc:
        with tc.tile_pool(name="persist", bufs=1) as persist:
            qk_all = persist.tile([P, 4, S], bf16)
            v_sb = persist.tile([P, NK, 256], f16)
            bqk_sb = persist.tile([P, 4], f32)
            bvr_sb = persist.tile([P, 256], f32)
            mkb_sb = persist.tile([P, NK], f32)
            ones_sb = persist.tile([P, 2, 64], f8e5)
            onesb_sb = persist.tile([P, 64], f16)
            dcmp_sb = persist.tile([P, 1], f32)
            if USE_FP8_PROJ:
                xq_sb = persist.tile([P, 4, 2, S], f8e4)
                wqk_sb = persist.tile([P, 4, 2, 512], f8e4)
                wv_sb = persist.tile([P, 4, 2, 256], f8e4)
            else:
                xq_sb = persist.tile([P, 8, S], bf16)
                wqk_sb = persist.tile([P, 8, 512], bf16)
                wv_sb = persist.tile([P, 8, 256], bf16)
            nc.sync.dma_start(out=bqk_sb[:], in_=bqk_d[:])
            nc.sync.dma_start(out=bvr_sb[:], in_=bvr_d[:])
            nc.sync.dma_start(out=mkb_sb[:], in_=mkb_d[:])
            nc.sync.dma_start(out=ones_sb[:], in_=ones_d[:])
            nc.sync.dma_start(out=onesb_sb[:], in_=onesb_d[:])
            nc.sync.dma_start(out=dcmp_sb[:], in_=dcmp_d[:])
            if USE_FP8_PROJ:
                for t in range(4):
                    nc.sync.dma_start(out=wqk_sb[:, t], in_=wqk_d[:, t])
                    nc.sync.dma_start(out=xq_sb[:, t], in_=xq_d[:, t])
                    nc.sync.dma_start(out=wv_sb[:, t], in_=wv_d[:, t])
            else:
                for t in range(8):
                    nc.sync.dma_start(out=wqk_sb[:, t], in_=wqk_d[:, t])
                    nc.sync.dma_start(out=xq_sb[:, t], in_=xq_d[:, t])
                    nc.sync.dma_start(out=wv_sb[:, t], in_=wv_d[:, t])

            with tc.tile_pool(name="estr", bufs=12) as ep, tc.tile_pool(
                name="astr", bufs=6
            ) as app, tc.tile_pool(name="ctp", bufs=6) as ctp, tc.tile_pool(
                name="small", bufs=1
            ) as smallp, tc.tile_pool(
                name="mm1ps", bufs=2, space="PSUM"
            ) as mm1p, tc.tile_pool(
                name="ops", bufs=1, space="PSUM"
            ) as pop, tc.tile_pool(
                name="rsps", bufs=1, space="PSUM"
            ) as prsp:

                def proj_qk(col, qb):
                    ps = mm1p.tile([P, S // 2], f32, tag="scT", name="pj")
                    if USE_FP8_PROJ:
                        for t in range(4):
                            nc.tensor.matmul(
                                ps[:, 0:512],
                                lhsT=wqk_sb[:, t, :, col * P : (col + 1) * P],
                                rhs=xq_sb[:, t, :, qb * 512 : (qb + 1) * 512],
                                perf_mode=DR,
                                start=(t == 0),
                                stop=(t == 3),
                            )
                    else:
                        for t in range(8):
                            nc.tensor.matmul(
                                ps[:, 0:512],
                                lhsT=wqk_sb[:, t, col * P : (col + 1) * P],
                                rhs=xq_sb[:, t, qb * 512 : (qb + 1) * 512],
                                start=(t == 0),
                                stop=(t == 7),
                            )
                    nc.vector.tensor_scalar_add(
                        out=qk_all[:, col, qb * 512 : (qb + 1) * 512],
                        in0=ps[:, 0:512],
                        scalar1=bqk_sb[:, col : col + 1],
                    )

                def proj_v(kt):
                    ps = mm1p.tile([P, S // 2], f32, tag="scT", name="pv")
                    if USE_FP8_PROJ:
                        for t in range(4):
                            nc.tensor.matmul(
                                ps[:, 0:256],
                                lhsT=xq_sb[:, t, :, kt * P : (kt + 1) * P],
                                rhs=wv_sb[:, t],
                                perf_mode=DR,
                                start=(t == 0),
                                stop=(t == 3),
                            )
                    else:
                        for t in range(8):
                            nc.tensor.matmul(
                                ps[:, 0:256],
                                lhsT=xq_sb[:, t, kt * P : (kt + 1) * P],
                                rhs=wv_sb[:, t],
                                start=(t == 0),
                                stop=(t == 7),
                            )
                    nc.vector.tensor_tensor(
                        v_sb[:, kt, :], ps[:, 0:256], bvr_sb[:], Alu.add
                    )

                def passA_strip(qh, pr, ks, eA, eB):
                    # scores^T [k=128, q=1024] per head into a 2-bank tile
                    pssA = mm1p.tile([P, S // 2], f32, tag="scT", name="pssA")
                    pssB = mm1p.tile([P, S // 2], f32, tag="scT", name="pssB")
                    for q2 in range(2):
                        qs = slice(qh * QH + q2 * 512, qh * QH + (q2 + 1) * 512)
                        os_ = slice(q2 * 512, (q2 + 1) * 512)
                        nc.tensor.matmul(
                            pssA[:, os_],
                            lhsT=qk_all[0:64, 2 * pr + 1, ks * P : (ks + 1) * P],
                            rhs=qk_all[0:64, 2 * pr, qs],
                            tile_position=(0, 0),
                            start=True,
                            stop=True,
                        )
                        nc.tensor.matmul(
                            pssB[:, os_],
                            lhsT=qk_all[64:128, 2 * pr + 1, ks * P : (ks + 1) * P],
                            rhs=qk_all[64:128, 2 * pr, qs],
                            tile_position=(64, 0),
                            start=True,
                            stop=True,
                        )
                    par = ks % 2
                    nc.scalar.activation(
                        eA[:, par, :],
                        pssA[:],
                        Act.Exp,
                        scale=EXPSCALE,
                        bias=mkb_sb[:, ks : ks + 1],
                    )
                    nc.scalar.activation(
                        eB[:, par, :],
                        pssB[:],
                        Act.Exp,
                        scale=EXPSCALE,
                        bias=mkb_sb[:, ks : ks + 1],
                    )

                def passB_strip(qh, pr, ks, eA, eB, po, prs):
                    h0, h1 = 2 * pr, 2 * pr + 1
                    par = ks % 2
                    ct = ctp.tile([P, QH], f16, tag="ct")
                    nc.sync.dma_start(
                        out=ct[:],
                        in_=ct_d[ks * P : (ks + 1) * P, qh * QH : (qh + 1) * QH],
                    )
                    aA = app.tile([P, QH], f16, tag="a")
                    aB = app.tile([P, QH], f16, tag="a")
                    # split the two big multiplies between DVE and Pool
                    nc.vector.tensor_tensor(aA[:], eA[:, par, :], ct[:], Alu.mult)
                    engB = nc.gpsimd if ks % 2 == 0 else nc.vector
                    engB.tensor_tensor(aB[:], eB[:, par, :], ct[:], Alu.mult)
                    st, sp = (ks == 0), (ks == NK - 1)
                    for q2 in range(2):
                        os_ = slice(q2 * 512, (q2 + 1) * 512)
                        nc.tensor.matmul(
                            po[0:64, os_],
                            lhsT=v_sb[:, ks, h0 * 64 : (h0 + 1) * 64],
                            rhs=aA[:, os_],
                            tile_position=(0, 0),
                            start=st,
                            stop=sp,
                        )
                        nc.tensor.matmul(
                            po[64:128, os_],
                            lhsT=v_sb[:, ks, h1 * 64 : (h1 + 1) * 64],
                            rhs=aB[:, os_],
                            tile_position=(0, 64),
                            start=st,
                            stop=sp,
                        )
                    # head B denominator: regular f16 ones matmul per strip
                    for q2 in range(2):
                        os_ = slice(q2 * 512, (q2 + 1) * 512)
                        nc.tensor.matmul(
                            prs[64:128, os_],
                            lhsT=onesb_sb[:],
                            rhs=eB[:, par, os_],
                            tile_position=(0, 64),
                            start=st,
                            stop=sp,
                        )
                    if par == 1:
                        # head A denominator: DoubleRow over the e5m2
                        # high-byte view of the fp16 e pair tile; 2 strips
                        # per call, tile (0,0) only (ISA restriction).
                        dst, dsp = (ks == 1), (ks == NK - 1)
                        ev = eA.bitcast(f8e5)
                        for q2 in range(2):
                            rv = ev[:, :, 2 * q2 * 512 + 1 : 2 * (q2 + 1) * 512 : 2]
                            nc.tensor.matmul(
                                prs[0:64, q2 * 512 : (q2 + 1) * 512],
                                lhsT=ones_sb[:],
                                rhs=rv,
                                perf_mode=DR,
                                start=dst,
                                stop=dsp,
                            )

                def finishB(qh, pr, po, prs):
                    rcs = smallp.tile([P, QH], f32, tag="rcs")
                    scr = smallp.tile([P, QH], f32, tag="scr")
                    nc.vector.reciprocal_approx_accurate(rcs[:], prs[:], scr[:])
                    ob = smallp.tile([P, QH], f32, tag="ob")
                    nc.vector.scalar_tensor_tensor(
                        ob[:], po[:], dcmp_sb[:, 0:1], rcs[:], Alu.mult, Alu.mult
                    )
                    nc.sync.dma_start(
                        out=out_d[pr * P : (pr + 1) * P, qh * QH : (qh + 1) * QH],
                        in_=ob[:],
                    )

                phases = [(qh, pr) for qh in range(2) for pr in range(2)]

                # projection schedule: (col, qb) for qk / ('v', kt) for v,
                # ordered by need-by iteration; drained ~1/iteration.
                projq = (
                    [("qk", 1, 1), ("v", 0), ("v", 1), ("qk", 1, 2)]
                    + [("v", 2), ("v", 3), ("qk", 1, 3), ("v", 4)]
                    + [("v", 5), ("v", 6), ("v", 7), ("v", 8), ("v", 9)]
                    + [("qk", 2, 0), ("qk", 2, 1), ("qk", 3, 0), ("v", 10)]
                    + [("v", 11), ("qk", 3, 1), ("v", 12), ("v", 13)]
                    + [("qk", 3, 2), ("v", 14), ("v", 15), ("qk", 3, 3)]
                    + [("qk", 0, 2), ("qk", 0, 3), ("qk", 2, 2), ("qk", 2, 3)]
                )
                deadlines = {
                    ("qk", 1, 1): 4, ("qk", 1, 2): 8, ("qk", 1, 3): 12,
                    ("qk", 2, 0): 16, ("qk", 2, 1): 16, ("qk", 3, 0): 16,
                    ("qk", 3, 1): 20, ("qk", 3, 2): 24, ("qk", 3, 3): 28,
                    ("qk", 0, 2): 32, ("qk", 0, 3): 32,
                    ("qk", 2, 2): 48, ("qk", 2, 3): 48,
                }
                for kt in range(NK):
                    deadlines[("v", kt)] = kt + BOFF

                def emit_proj(item):
                    if item[0] == "qk":
                        proj_qk(item[1], item[2])
                    else:
                        proj_v(item[1])

                # prologue: Q(pair0, qh0) + first K(pair0) block
                proj_qk(0, 0)
                proj_qk(0, 1)
                proj_qk(1, 0)

                es_all = {}
                bstate = {}
                pptr = [0]
                NITER = 4 * NK + BOFF
                for g in range(NITER):
                    ph, ks = g // NK, g % NK
                    if g < 4 * NK:
                        qh, pr = phases[ph]
                        if ks % 2 == 0:
                            eA = ep.tile([P, 2, QH], f16, tag="e", name="eA")
                            eB = ep.tile([P, 2, QH], f16, tag="e", name="eB")
                            es_all.setdefault(ph, []).append((eA, eB))
                        eA, eB = es_all[ph][ks // 2]
                        passA_strip(qh, pr, ks, eA, eB)
                    # drain projection queue (deadline-forced, else 1/iter)
                    emitted = 0
                    while pptr[0] < len(projq) and (
                        deadlines[projq[pptr[0]]] <= g + 2 or emitted == 0
                    ):
                        emit_proj(projq[pptr[0]])
                        pptr[0] += 1
                        emitted += 1
                        if emitted >= 2 and not (
                            pptr[0] < len(projq)
                            and deadlines[projq[pptr[0]]] <= g + 2
                        ):
                            break
                    bg = g - BOFF
                    if bg >= 0:
                        bph, bks = bg // NK, bg % NK
                        bqh, bpr = phases[bph]
                        if bks == 0:
                            b_po = pop.tile([P, QH], f32, tag="po", name="po")
                            b_prs = prsp.tile([P, QH], f32, tag="prs", name="prs")
                            bstate[bph] = (b_po, b_prs)
                        b_po, b_prs = bstate[bph]
                        eA, eB = es_all[bph][bks // 2]
                        passB_strip(bqh, bpr, bks, eA, eB, b_po, b_prs)
                        if bks == NK - 1:
                            finishB(bqh, bpr, b_po, b_prs)
                            del bstate[bph]
                            del es_all[bph]

    nc.finalize()
    return nc


def _get_program():
    if "nc" not in _prog_cache:
        _prog_cache["nc"] = _build_program()
    return _prog_cache["nc"]


def kernel(x, attention_mask, C_prior, Wq, bq, Wk, bk, Wv, bv):
    from concourse.bass_utils import run_bass_kernel_spmd

    x = np.asarray(x, dtype=np.float32)
    attention_mask = np.asarray(attention_mask)
    C_prior = np.asarray(C_prior, dtype=np.float32)
    Wq = np.asarray(Wq, dtype=np.float32)
    Wk = np.asarray(Wk, dtype=np.float32)
    Wv = np.asarray(Wv, dtype=np.float32)
    bq = np.asarray(bq, dtype=np.float32)
    bk = np.asarray(bk, dtype=np.float32)
    bv = np.asarray(bv, dtype=np.float32)
    bf = ml_dtypes.bfloat16
    f8e4 = ml_dtypes.float8_e4m3fn
    f8e5 = ml_dtypes.float8_e5m2

    WqT, WkT, WvT = Wq.T, Wk.T, Wv.T  # [in D, out D]
    maskf = attention_mask.astype(np.float32)  # [B, S]

    def dr_pack(a):
        # [D, M] -> [128, 4, 2, M] with d = t*256 + i*128 + p
        Dd, M = a.shape
        return np.ascontiguousarray(
            a.reshape(4, 2, P, M).transpose(2, 0, 1, 3)
        )

    in_maps = []
    for c in range(NCORES):
        b, hg = c // 4, c % 4
        heads = [4 * hg + i for i in range(HEADS_PER_CORE)]

        wqk = np.empty((D, 512), np.float32)
        bqk = np.zeros((P, 4), np.float32)
        for pr in range(2):
            h0, h1 = heads[2 * pr], heads[2 * pr + 1]
            wqk[:, (2 * pr) * P : (2 * pr) * P + 64] = WqT[:, h0 * 64 : h0 * 64 + 64]
            wqk[:, (2 * pr) * P + 64 : (2 * pr + 1) * P] = WqT[
                :, h1 * 64 : h1 * 64 + 64
            ]
            wqk[:, (2 * pr + 1) * P : (2 * pr + 1) * P + 64] = WkT[
                :, h0 * 64 : h0 * 64 + 64
            ]
            wqk[:, (2 * pr + 1) * P + 64 : (2 * pr + 2) * P] = WkT[
                :, h1 * 64 : h1 * 64 + 64
            ]
            bqk[0:64, 2 * pr] = bq[h0 * 64 : h0 * 64 + 64]
            bqk[64:128, 2 * pr] = bq[h1 * 64 : h1 * 64 + 64]
            bqk[0:64, 2 * pr + 1] = bk[h0 * 64 : h0 * 64 + 64]
            bqk[64:128, 2 * pr + 1] = bk[h1 * 64 : h1 * 64 + 64]

        wv = np.ascontiguousarray(WvT[:, heads[0] * 64 : (heads[-1] + 1) * 64])
        bvr_v = bv[heads[0] * 64 : (heads[-1] + 1) * 64]

        xT = np.ascontiguousarray(x[b].T)  # [D, S]
        if USE_FP8_PROJ:
            xq = dr_pack(xT).astype(f8e4)
            wqk_in = dr_pack(wqk * W8).astype(f8e4)
            wv_in = dr_pack(wv * W8).astype(f8e4)
            bqk_in = bqk * W8
            bvr_in = np.ascontiguousarray(
                np.broadcast_to(bvr_v[None, :] * W8, (P, 256))
            ).astype(np.float32)
        else:
            xq = xT.reshape(8, P, S).transpose(1, 0, 2).astype(bf)
            xq = np.ascontiguousarray(xq)
            wqk_in = np.ascontiguousarray(
                wqk.reshape(8, P, 512).transpose(1, 0, 2)
            ).astype(bf)
            wv_in = np.ascontiguousarray(
                wv.reshape(8, P, 256).transpose(1, 0, 2)
            ).astype(bf)
            bqk_in = bqk
            bvr_in = np.ascontiguousarray(
                np.broadcast_to(bvr_v[None, :], (P, 256))
            ).astype(np.float32)

        m = maskf[b]  # [S]
        ct = (C_prior[b].T * m[:, None]).astype(np.float16)  # [S(k), S(q)]
        # exp bias per (k-partition, strip): round-to-nearest offset + mask
        mkb = np.full((P, NK), ROUND_BIAS, np.float32)
        mkb += np.where(m.reshape(NK, P).T > 0, 0.0, -1e9).astype(np.float32)
        vs = W8 if USE_FP8_PROJ else 1.0
        ones3 = np.full((P, 2, 64), vs, f8e5)
        onesb = np.full((P, 64), vs, np.float16)
        dcmp = np.ones((P, 1), np.float32)
        dcmp[0:64] = E5_TRUNC_RATIO

        in_maps.append(
            {
                "xq": xq,
                "wqk": wqk_in,
                "wv": wv_in,
                "bqk": bqk_in,
                "bvr": bvr_in,
                "ct": ct,
                "mkb": mkb,
                "ones3": ones3,
                "onesb": onesb,
                "dcmp": dcmp,
            }
        )

    nc = _get_program()
    trace = bool(int(os.environ.get("BASS_KERNEL_TRACE", "0")))
    res = run_bass_kernel_spmd(nc, in_maps, list(range(NCORES)), trace=trace)
    if trace:
        print(f"HW exec time: {res.exec_time_ns} ns")
        _prog_cache["last_exec_time_ns"] = res.exec_time_ns
        _prog_cache["last_trace"] = res.instructions_and_trace

    out = np.empty((B, S, D), np.float32)
    for c in range(NCORES):
        b, hg = c // 4, c % 4
        co = res.results[c]["out"]  # [256, S]
        for i in range(HEADS_PER_CORE):
            h = 4 * hg + i
            out[b, :, h * 64 : (h + 1) * 64] = co[i * 64 : (i + 1) * 64, :].T
    return out


# revision 15
# speedup vs baseline: 1.0502x; 1.0502x over previous
"""Trainium2 Bass kernel for BertSelfAttention with C_prior multiply.

Reference (per batch b):
  q/k/v = x @ W{q,k,v}.T + b{q,k,v}            -> [S, D], H=16 heads of W=64
  scores = q k^T / sqrt(W); mask; softmax over k
  attn = softmax(scores) * C_prior[b]
  out = attn @ v                               -> [B, S, D]

Shapes: B=2, S=2048, D=1024, H=16, W=64.
Sharding: 8 cores; core c owns batch b=c//4 and 4 heads (hg=c%4).

Measured HW model driving the design: PE matmul time ~ max(PSUM f32
writes / 265G/s, rhs cols / 3.9G/s); Act = 128 lanes @1.2GHz; DVE gets
2x on 16-bit SBUF ops; Pool engine (nc.gpsimd) is a second 1.2GHz
vector engine.

Design vs the naive version:
  - Projections in fp8e4m3 DoubleRow: contraction 256 per step -> 4
    PSUM accumulation passes instead of 8 (proj PE time halves).
    Weights are pre-scaled x8 on host (so they sit in e4m3 normal
    range); compensated via the exp scale (q,k) and the broadcast
    constant (v).
  - scores^T [k,q] per (strip, head) land in a [128,1024] PSUM tile
    (2 banks); ONE exp per (strip, head) -> halves Act instruction count.
  - e is stored fp16. The softmax denominator is computed by a
    DoubleRow fp8e5m2 matmul whose rhs is the strided high-byte view
    of the fp16 e tile (fp16 high byte == e5m2 truncation). exp is
    biased by ln(1.0625) (half an m2 ulp) so truncation becomes
    round-to-nearest; the 1.0625 cancels between numerator and
    denominator. Thin [1,512] outputs: denominator streams 2 strips
    per call and writes almost nothing.
  - 1/denom via thin reciprocal (DVE) then broadcast up to 128
    partitions with a tiny f32r matmul whose host constant also holds
    the 1/8 fp8-V compensation.
  - mask folded into the per-partition exp bias; attn*C multiplies
    split between DVE and Pool; projection bias-adds (PSUM->SBUF
    moves) on Pool.
"""

import math
import os

import numpy as np
import ml_dtypes

B, S, D, H, W = 2, 2048, 1024, 16, 64
NCORES = 8
HEADS_PER_CORE = 4
P = 128
QH = S // 2  # q processed in two halves of 1024
NK = S // P  # 16 k-strips
BOFF = 6  # pass B trails pass A by this many strips

USE_FP8_PROJ = False
ROUND_BIAS = 0.0  # no pre-scale: truncation bias is scale-invariant
E5_TRUNC_RATIO = 0.91578  # E[e5m2_trunc(x)/x] for exp(N(0,1)) mantissas
W8 = 8.0  # host pre-scale on all projection weights (fp8 range)

_prog_cache = {}


def _build_program():
    import concourse.mybir as mybir
    import concourse.tile as tile
    from concourse import bacc

    dt = mybir.dt
    f32, bf16, f16 = dt.float32, dt.bfloat16, dt.float16
    f8e4, f8e5, f32r = dt.float8e4, dt.float8e5, dt.float32r
    Alu = mybir.AluOpType
    Act = mybir.ActivationFunctionType
    DR = mybir.MatmulPerfMode.DoubleRow

    nc = bacc.Bacc("TRN2", target_bir_lowering=False)

    if USE_FP8_PROJ:
        # x/W laid out for DoubleRow: contraction d = t*256 + i*128 + p
        xq_d = nc.declare_dram_parameter("xq", [P, 4, 2, S], f8e4, isOutput=False)
        wqk_d = nc.declare_dram_parameter("wqk", [P, 4, 2, 512], f8e4, isOutput=False)
        wv_d = nc.declare_dram_parameter("wv", [P, 4, 2, 256], f8e4, isOutput=False)
    else:
        xq_d = nc.declare_dram_parameter("xq", [P, 8, S], bf16, isOutput=False)
        wqk_d = nc.declare_dram_parameter("wqk", [P, 8, 512], bf16, isOutput=False)
        wv_d = nc.declare_dram_parameter("wv", [P, 8, 256], bf16, isOutput=False)
    bqk_d = nc.declare_dram_parameter("bqk", [P, 4], f32, isOutput=False)
    bvr_d = nc.declare_dram_parameter("bvr", [P, 256], f32, isOutput=False)
    ct_d = nc.declare_dram_parameter("ct", [S, S], f16, isOutput=False)
    mkb_d = nc.declare_dram_parameter("mkb", [P, NK], f32, isOutput=False)
    ones_d = nc.declare_dram_parameter("ones3", [P, 2, 64], f8e5, isOutput=False)
    onesb_d = nc.declare_dram_parameter("onesb", [P, 64], f16, isOutput=False)
    dcmp_d = nc.declare_dram_parameter("dcmp", [P, 1], f32, isOutput=False)
    out_d = nc.declare_dram_parameter("out", [256, S], f32, isOutput=True)

    EXPSCALE = 0.125 / (W8 * W8) if USE_FP8_PROJ else 0.125

    with tile.TileContext(nc) as tc:
        with tc.tile_pool(name="persist", bufs=1) as persist:
            qk_all = persist.tile([P, 4, S], bf16)
            v_sb = persist.tile([P, NK, 256], f16)
            bqk_sb = persist.tile([P, 4], f32)
            bvr_sb = persist.tile([P, 256], f32)
            mkb_sb = persist.tile([P, NK], f32)
            ones_sb = persist.tile([P, 2, 64], f8e5)
            onesb_sb = persist.tile([P, 64], f16)
            dcmp_sb = persist.tile([P, 1], f32)
            if USE_FP8_PROJ:
                xq_sb = persist.tile([P, 4, 2, S], f8e4)
                wqk_sb = persist.tile([P, 4, 2, 512], f8e4)
                wv_sb = persist.tile([P, 4, 2, 256], f8e4)
            else:
                xq_sb = persist.tile([P, 8, S], bf16)
                wqk_sb = persist.tile([P, 8, 512], bf16)
                wv_sb = persist.tile([P, 8, 256], bf16)
            nc.sync.dma_start(out=bqk_sb[:], in_=bqk_d[:])
            nc.sync.dma_start(out=bvr_sb[:], in_=bvr_d[:])
            nc.sync.dma_start(out=mkb_sb[:], in_=mkb_d[:])
            nc.sync.dma_start(out=ones_sb[:], in_=ones_d[:])
            nc.sync.dma_start(out=onesb_sb[:], in_=onesb_d[:])
            nc.sync.dma_start(out=dcmp_sb[:], in_=dcmp_d[:])
            if USE_FP8_PROJ:
                for t in range(4):
                    nc.sync.dma_start(out=wqk_sb[:, t], in_=wqk_d[:, t])
                    nc.sync.dma_start(out=xq_sb[:, t], in_=xq_d[:, t])
                    nc.sync.dma_start(out=wv_sb[:, t], in_=wv_d[:, t])
            else:
                for t in range(8):
                    nc.sync.dma_start(out=wqk_sb[:, t], in_=wqk_d[:, t])
                    nc.sync.dma_start(out=xq_sb[:, t], in_=xq_d[:, t])
                    nc.sync.dma_start(out=wv_sb[:, t], in_=wv_d[:, t])

            with tc.tile_pool(name="estr", bufs=12) as ep, tc.tile_pool(
                name="astr", bufs=6
            ) as app, tc.tile_pool(name="ctp", bufs=6) as ctp, tc.tile_pool(
                name="small", bufs=1
            ) as smallp, tc.tile_pool(
                name="mm1ps", bufs=2, space="PSUM"
            ) as mm1p, tc.tile_pool(
                name="ops", bufs=1, space="PSUM"
            ) as pop, tc.tile_pool(
                name="rsps", bufs=1, space="PSUM"
            ) as prsp:

                def proj_qk(col, qb):
                    ps = mm1p.tile([P, S // 2], f32, tag="scT", name="pj")
                    if USE_FP8_PROJ:
                        for t in range(4):
                            nc.tensor.matmul(
                                ps[:, 0:512],
                                lhsT=wqk_sb[:, t, :, col * P : (col + 1) * P],
                                rhs=xq_sb[:, t, :, qb * 512 : (qb + 1) * 512],
                                perf_mode=DR,
                                start=(t == 0),
                                stop=(t == 3),
                            )
                    else:
                        for t in range(8):
                            nc.tensor.matmul(
                                ps[:, 0:512],
                                lhsT=wqk_sb[:, t, col * P : (col + 1) * P],
                                rhs=xq_sb[:, t, qb * 512 : (qb + 1) * 512],
                                start=(t == 0),
                                stop=(t == 7),
                            )
                    nc.vector.tensor_scalar_add(
                        out=qk_all[:, col, qb * 512 : (qb + 1) * 512],
                        in0=ps[:, 0:512],
                        scalar1=bqk_sb[:, col : col + 1],
                    )

                def proj_v(kt):
                    ps = mm1p.tile([P, S // 2], f32, tag="scT", name="pv")
                    if USE_FP8_PROJ:
                        for t in range(4):
                            nc.tensor.matmul(
                                ps[:, 0:256],
                                lhsT=xq_sb[:, t, :, kt * P : (kt + 1) * P],
                                rhs=wv_sb[:, t],
                                perf_mode=DR,
                                start=(t == 0),
                                stop=(t == 3),
                            )
                    else:
                        for t in range(8):
                            nc.tensor.matmul(
                                ps[:, 0:256],
                                lhsT=xq_sb[:, t, kt * P : (kt + 1) * P],
                                rhs=wv_sb[:, t],
                                start=(t == 0),
                                stop=(t == 7),
                            )
                    nc.vector.tensor_tensor(
                        v_sb[:, kt, :], ps[:, 0:256], bvr_sb[:], Alu.add
                    )

                def passA_strip(qh, pr, ks, eA, eB):
                    # scores^T [k=128, q=1024] per head into a 2-bank tile
                    pssA = mm1p.tile([P, S // 2], f32, tag="scT", name="pssA")
                    pssB = mm1p.tile([P, S // 2], f32, tag="scT", name="pssB")
                    for q2 in range(2):
                        qs = slice(qh * QH + q2 * 512, qh * QH + (q2 + 1) * 512)
                        os_ = slice(q2 * 512, (q2 + 1) * 512)
                        nc.tensor.matmul(
                            pssA[:, os_],
                            lhsT=qk_all[0:64, 2 * pr + 1, ks * P : (ks + 1) * P],
                            rhs=qk_all[0:64, 2 * pr, qs],
                            tile_position=(0, 0),
                            start=True,
                            stop=True,
                        )
                        nc.tensor.matmul(
                            pssB[:, os_],
                            lhsT=qk_all[64:128, 2 * pr + 1, ks * P : (ks + 1) * P],
                            rhs=qk_all[64:128, 2 * pr, qs],
                            tile_position=(64, 0),
                            start=True,
                            stop=True,
                        )
                    par = ks % 2
                    nc.scalar.activation(
                        eA[:, par, :],
                        pssA[:],
                        Act.Exp,
                        scale=EXPSCALE,
                        bias=mkb_sb[:, ks : ks + 1],
                    )
                    nc.scalar.activation(
                        eB[:, par, :],
                        pssB[:],
                        Act.Exp,
                        scale=EXPSCALE,
                        bias=mkb_sb[:, ks : ks + 1],
                    )

                def passB_strip(qh, pr, ks, eA, eB, po, prs):
                    h0, h1 = 2 * pr, 2 * pr + 1
                    par = ks % 2
                    ct = ctp.tile([P, QH], f16, tag="ct")
                    nc.sync.dma_start(
                        out=ct[:],
                        in_=ct_d[ks * P : (ks + 1) * P, qh * QH : (qh + 1) * QH],
                    )
                    aA = app.tile([P, QH], f16, tag="a")
                    aB = app.tile([P, QH], f16, tag="a")
                    # split the two big multiplies between DVE and Pool
                    nc.vector.tensor_tensor(aA[:], eA[:, par, :], ct[:], Alu.mult)
                    engB = nc.gpsimd if ks % 2 == 0 else nc.vector
                    engB.tensor_tensor(aB[:], eB[:, par, :], ct[:], Alu.mult)
                    st, sp = (ks == 0), (ks == NK - 1)
                    for q2 in range(2):
                        os_ = slice(q2 * 512, (q2 + 1) * 512)
                        nc.tensor.matmul(
                            po[0:64, os_],
                            lhsT=v_sb[:, ks, h0 * 64 : (h0 + 1) * 64],
                            rhs=aA[:, os_],
                            tile_position=(0, 0),
                            start=st,
                            stop=sp,
                        )
                        nc.tensor.matmul(
                            po[64:128, os_],
                            lhsT=v_sb[:, ks, h1 * 64 : (h1 + 1) * 64],
                            rhs=aB[:, os_],
                            tile_position=(0, 64),
                            start=st,
                            stop=sp,
                        )
                    # denominators: regular f16 ones matmuls per strip
                    for q2 in range(2):
                        os_ = slice(q2 * 512, (q2 + 1) * 512)
                        nc.tensor.matmul(
                            prs[0:64, os_],
                            lhsT=onesb_sb[:],
                            rhs=eA[:, par, os_],
                            tile_position=(0, 0),
                            start=st,
                            stop=sp,
                        )
                        nc.tensor.matmul(
                            prs[64:128, os_],
                            lhsT=onesb_sb[:],
                            rhs=eB[:, par, os_],
                            tile_position=(0, 64),
                            start=st,
                            stop=sp,
                        )

                def finishB(qh, pr, po, prs):
                    rcs = smallp.tile([P, QH], f32, tag="rcs")
                    scr = smallp.tile([P, QH], f32, tag="scr")
                    nc.vector.reciprocal_approx_accurate(rcs[:], prs[:], scr[:])
                    ob = smallp.tile([P, QH], f32, tag="ob")
                    nc.vector.scalar_tensor_tensor(
                        ob[:], po[:], dcmp_sb[:, 0:1], rcs[:], Alu.mult, Alu.mult
                    )
                    nc.sync.dma_start(
                        out=out_d[pr * P : (pr + 1) * P, qh * QH : (qh + 1) * QH],
                        in_=ob[:],
                    )

                phases = [(qh, pr) for qh in range(2) for pr in range(2)]

                # projection schedule: (col, qb) for qk / ('v', kt) for v,
                # ordered by need-by iteration; drained ~1/iteration.
                projq = (
                    [("qk", 1, 1), ("v", 0), ("v", 1), ("qk", 1, 2)]
                    + [("v", 2), ("v", 3), ("qk", 1, 3), ("v", 4)]
                    + [("v", 5), ("v", 6), ("v", 7), ("v", 8), ("v", 9)]
                    + [("qk", 2, 0), ("qk", 2, 1), ("qk", 3, 0), ("v", 10)]
                    + [("v", 11), ("qk", 3, 1), ("v", 12), ("v", 13)]
                    + [("qk", 3, 2), ("v", 14), ("v", 15), ("qk", 3, 3)]
                    + [("qk", 0, 2), ("qk", 0, 3), ("qk", 2, 2), ("qk", 2, 3)]
                )
                deadlines = {
                    ("qk", 1, 1): 4, ("qk", 1, 2): 8, ("qk", 1, 3): 12,
                    ("qk", 2, 0): 16, ("qk", 2, 1): 16, ("qk", 3, 0): 16,
                    ("qk", 3, 1): 20, ("qk", 3, 2): 24, ("qk", 3, 3): 28,
                    ("qk", 0, 2): 32, ("qk", 0, 3): 32,
                    ("qk", 2, 2): 48, ("qk", 2, 3): 48,
                }
                for kt in range(NK):
                    deadlines[("v", kt)] = kt + BOFF

                def emit_proj(item):
                    if item[0] == "qk":
                        proj_qk(item[1], item[2])
                    else:
                        proj_v(item[1])

                # prologue: Q(pair0, qh0) + first K(pair0) block
                proj_qk(0, 0)
                proj_qk(0, 1)
                proj_qk(1, 0)

                es_all = {}
                bstate = {}
                pptr = [0]
                NITER = 4 * NK + BOFF
                for g in range(NITER):
                    ph, ks = g // NK, g % NK
                    if g < 4 * NK:
                        qh, pr = phases[ph]
                        if ks % 2 == 0:
                            eA = ep.tile([P, 2, QH], f16, tag="e", name="eA")
                            eB = ep.tile([P, 2, QH], f16, tag="e", name="eB")
                            es_all.setdefault(ph, []).append((eA, eB))
                        eA, eB = es_all[ph][ks // 2]
                        passA_strip(qh, pr, ks, eA, eB)
                    # drain projection queue (deadline-forced, else 1/iter)
                    emitted = 0
                    while pptr[0] < len(projq) and (
                        deadlines[projq[pptr[0]]] <= g + 2 or emitted == 0
                    ):
                        emit_proj(projq[pptr[0]])
                        pptr[0] += 1
                        emitted += 1
                        if emitted >= 2 and not (
                            pptr[0] < len(projq)
                            and deadlines[projq[pptr[0]]] <= g + 2
                        ):
                            break
                    bg = g - BOFF
                    if bg >= 0:
                        bph, bks = bg // NK, bg % NK
                        bqh, bpr = phases[bph]
                        if bks == 0:
                            b_po = pop.tile([P, QH], f32, tag="po", name="po")
                            b_prs = prsp.tile([P, QH], f32, tag="prs", name="prs")
                            bstate[bph] = (b_po, b_prs)
                        b_po, b_prs = bstate[bph]
                        eA, eB = es_all[bph][bks // 2]
                        passB_strip(bqh, bpr, bks, eA, eB, b_po, b_prs)
                        if bks == NK - 1:
                            finishB(bqh, bpr, b_po, b_prs)
                            del bstate[bph]
                            del es_all[bph]

    nc.finalize()
    return nc


def _get_program():
    if "nc" not in _prog_cache:
        _prog_cache["nc"] = _build_program()
    return _prog_cache["nc"]


def kernel(x, attention_mask, C_prior, Wq, bq, Wk, bk, Wv, bv):
    from concourse.bass_utils import run_bass_kernel_spmd

    x = np.asarray(x, dtype=np.float32)
    attention_mask = np.asarray(attention_mask)
    C_prior = np.asarray(C_prior, dtype=np.float32)
    Wq = np.asarray(Wq, dtype=np.float32)
    Wk = np.asarray(Wk, dtype=np.float32)
    Wv = np.asarray(Wv, dtype=np.float32)
    bq = np.asarray(bq, dtype=np.float32)
    bk = np.asarray(bk, dtype=np.float32)
    bv = np.asarray(bv, dtype=np.float32)
    bf = ml_dtypes.bfloat16
    f8e4 = ml_dtypes.float8_e4m3fn
    f8e5 = ml_dtypes.float8_e5m2

    WqT, WkT, WvT = Wq.T, Wk.T, Wv.T  # [in D, out D]
    maskf = attention_mask.astype(np.float32)  # [B, S]

    def dr_pack(a):
        # [D, M] -> [128, 4, 2, M] with d = t*256 + i*128 + p
        Dd, M = a.shape
        return np.ascontiguousarray(
            a.reshape(4, 2, P, M).transpose(2, 0, 1, 3)
        )

    in_maps = []
    for c in range(NCORES):
        b, hg = c // 4, c % 4
        heads = [4 * hg + i for i in range(HEADS_PER_CORE)]

        wqk = np.empty((D, 512), np.float32)
        bqk = np.zeros((P, 4), np.float32)
        for pr in range(2):
            h0, h1 = heads[2 * pr], heads[2 * pr + 1]
            wqk[:, (2 * pr) * P : (2 * pr) * P + 64] = WqT[:, h0 * 64 : h0 * 64 + 64]
            wqk[:, (2 * pr) * P + 64 : (2 * pr + 1) * P] = WqT[
                :, h1 * 64 : h1 * 64 + 64
            ]
            wqk[:, (2 * pr + 1) * P : (2 * pr + 1) * P + 64] = WkT[
                :, h0 * 64 : h0 * 64 + 64
            ]
            wqk[:, (2 * pr + 1) * P + 64 : (2 * pr + 2) * P] = WkT[
                :, h1 * 64 : h1 * 64 + 64
            ]
            bqk[0:64, 2 * pr] = bq[h0 * 64 : h0 * 64 + 64]
            bqk[64:128, 2 * pr] = bq[h1 * 64 : h1 * 64 + 64]
            bqk[0:64, 2 * pr + 1] = bk[h0 * 64 : h0 * 64 + 64]
            bqk[64:128, 2 * pr + 1] = bk[h1 * 64 : h1 * 64 + 64]

        wv = np.ascontiguousarray(WvT[:, heads[0] * 64 : (heads[-1] + 1) * 64])
        bvr_v = bv[heads[0] * 64 : (heads[-1] + 1) * 64]

        xT = np.ascontiguousarray(x[b].T)  # [D, S]
        if USE_FP8_PROJ:
            xq = dr_pack(xT).astype(f8e4)
            wqk_in = dr_pack(wqk * W8).astype(f8e4)
            wv_in = dr_pack(wv * W8).astype(f8e4)
            bqk_in = bqk * W8
            bvr_in = np.ascontiguousarray(
                np.broadcast_to(bvr_v[None, :] * W8, (P, 256))
            ).astype(np.float32)
        else:
            xq = xT.reshape(8, P, S).transpose(1, 0, 2).astype(bf)
            xq = np.ascontiguousarray(xq)
            wqk_in = np.ascontiguousarray(
                wqk.reshape(8, P, 512).transpose(1, 0, 2)
            ).astype(bf)
            wv_in = np.ascontiguousarray(
                wv.reshape(8, P, 256).transpose(1, 0, 2)
            ).astype(bf)
            bqk_in = bqk
            bvr_in = np.ascontiguousarray(
                np.broadcast_to(bvr_v[None, :], (P, 256))
            ).astype(np.float32)

        m = maskf[b]  # [S]
        ct = (C_prior[b].T * m[:, None]).astype(np.float16)  # [S(k), S(q)]
        # exp bias per (k-partition, strip): round-to-nearest offset + mask
        mkb = np.full((P, NK), ROUND_BIAS, np.float32)
        mkb += np.where(m.reshape(NK, P).T > 0, 0.0, -1e9).astype(np.float32)
        vs = W8 if USE_FP8_PROJ else 1.0
        ones3 = np.full((P, 2, 64), vs, f8e5)
        onesb = np.full((P, 64), vs, np.float16)
        dcmp = np.ones((P, 1), np.float32)

        in_maps.append(
            {
                "xq": xq,
                "wqk": wqk_in,
                "wv": wv_in,
                "bqk": bqk_in,
                "bvr": bvr_in,
                "ct": ct,
                "mkb": mkb,
                "ones3": ones3,
                "onesb": onesb,
                "dcmp": dcmp,
            }
        )

    nc = _get_program()
    trace = bool(int(os.environ.get("BASS_KERNEL_TRACE", "0")))
    res = run_bass_kernel_spmd(nc, in_maps, list(range(NCORES)), trace=trace)
    if trace:
        print(f"HW exec time: {res.exec_time_ns} ns")
        _prog_cache["last_exec_time_ns"] = res.exec_time_ns
        _prog_cache["last_trace"] = res.instructions_and_trace

    out = np.empty((B, S, D), np.float32)
    for c in range(NCORES):
        b, hg = c // 4, c % 4
        co = res.results[c]["out"]  # [256, S]
        for i in range(HEADS_PER_CORE):
            h = 4 * hg + i
            out[b, :, h * 64 : (h + 1) * 64] = co[i * 64 : (i + 1) * 64, :].T
    return out


# revision 16
# speedup vs baseline: 1.0536x; 1.0032x over previous
"""Trainium2 Bass kernel for BertSelfAttention with C_prior multiply.

Reference (per batch b):
  q/k/v = x @ W{q,k,v}.T + b{q,k,v}            -> [S, D], H=16 heads of W=64
  scores = q k^T / sqrt(W); mask; softmax over k
  attn = softmax(scores) * C_prior[b]
  out = attn @ v                               -> [B, S, D]

Shapes: B=2, S=2048, D=1024, H=16, W=64.
Sharding: 8 cores; core c owns batch b=c//4 and 4 heads (hg=c%4).

Measured HW model driving the design: PE matmul time ~ max(PSUM f32
writes / 265G/s, rhs cols / 3.9G/s); Act = 128 lanes @1.2GHz; DVE gets
2x on 16-bit SBUF ops; Pool engine (nc.gpsimd) is a second 1.2GHz
vector engine.

Design vs the naive version:
  - Projections in fp8e4m3 DoubleRow: contraction 256 per step -> 4
    PSUM accumulation passes instead of 8 (proj PE time halves).
    Weights are pre-scaled x8 on host (so they sit in e4m3 normal
    range); compensated via the exp scale (q,k) and the broadcast
    constant (v).
  - scores^T [k,q] per (strip, head) land in a [128,1024] PSUM tile
    (2 banks); ONE exp per (strip, head) -> halves Act instruction count.
  - e is stored fp16. The softmax denominator is computed by a
    DoubleRow fp8e5m2 matmul whose rhs is the strided high-byte view
    of the fp16 e tile (fp16 high byte == e5m2 truncation). exp is
    biased by ln(1.0625) (half an m2 ulp) so truncation becomes
    round-to-nearest; the 1.0625 cancels between numerator and
    denominator. Thin [1,512] outputs: denominator streams 2 strips
    per call and writes almost nothing.
  - 1/denom via thin reciprocal (DVE) then broadcast up to 128
    partitions with a tiny f32r matmul whose host constant also holds
    the 1/8 fp8-V compensation.
  - mask folded into the per-partition exp bias; attn*C multiplies
    split between DVE and Pool; projection bias-adds (PSUM->SBUF
    moves) on Pool.
"""

import math
import os

import numpy as np
import ml_dtypes

B, S, D, H, W = 2, 2048, 1024, 16, 64
NCORES = 8
HEADS_PER_CORE = 4
P = 128
QH = S // 2  # q processed in two halves of 1024
NK = S // P  # 16 k-strips
BOFF = 6  # pass B trails pass A by this many strips

USE_FP8_PROJ = False
ROUND_BIAS = 0.0  # no pre-scale: truncation bias is scale-invariant
E5_TRUNC_RATIO = 0.91578  # E[e5m2_trunc(x)/x] for exp(N(0,1)) mantissas
W8 = 8.0  # host pre-scale on all projection weights (fp8 range)

_prog_cache = {}


def _build_program():
    import concourse.mybir as mybir
    import concourse.tile as tile
    from concourse import bacc

    dt = mybir.dt
    f32, bf16, f16 = dt.float32, dt.bfloat16, dt.float16
    f8e4, f8e5, f32r = dt.float8e4, dt.float8e5, dt.float32r
    Alu = mybir.AluOpType
    Act = mybir.ActivationFunctionType
    DR = mybir.MatmulPerfMode.DoubleRow

    nc = bacc.Bacc("TRN2", target_bir_lowering=False)

    if USE_FP8_PROJ:
        # x/W laid out for DoubleRow: contraction d = t*256 + i*128 + p
        xq_d = nc.declare_dram_parameter("xq", [P, 4, 2, S], f8e4, isOutput=False)
        wqk_d = nc.declare_dram_parameter("wqk", [P, 4, 2, 512], f8e4, isOutput=False)
        wv_d = nc.declare_dram_parameter("wv", [P, 4, 2, 256], f8e4, isOutput=False)
    else:
        xq_d = nc.declare_dram_parameter("xq", [P, 8, S], bf16, isOutput=False)
        wqk_d = nc.declare_dram_parameter("wqk", [P, 8, 512], bf16, isOutput=False)
        wv_d = nc.declare_dram_parameter("wv", [P, 8, 256], bf16, isOutput=False)
    bqk_d = nc.declare_dram_parameter("bqk", [P, 4], f32, isOutput=False)
    bvr_d = nc.declare_dram_parameter("bvr", [P, 256], f32, isOutput=False)
    ct_d = nc.declare_dram_parameter("ct", [S, S], f16, isOutput=False)
    mkb_d = nc.declare_dram_parameter("mkb", [P, NK], f32, isOutput=False)
    ones_d = nc.declare_dram_parameter("ones3", [P, 2, 64], f8e5, isOutput=False)
    onesb_d = nc.declare_dram_parameter("onesb", [P, 64], f16, isOutput=False)
    dcmp_d = nc.declare_dram_parameter("dcmp", [P, 1], f32, isOutput=False)
    out_d = nc.declare_dram_parameter("out", [256, S], f32, isOutput=True)

    EXPSCALE = 0.125 / (W8 * W8) if USE_FP8_PROJ else 0.125

    with tile.TileContext(nc) as tc:
        with tc.tile_pool(name="persist", bufs=1) as persist:
            qk_all = persist.tile([P, 4, S], bf16)
            v_sb = persist.tile([P, NK, 256], f16)
            bqk_sb = persist.tile([P, 4], f32)
            bvr_sb = persist.tile([P, 256], f32)
            mkb_sb = persist.tile([P, NK], f32)
            ones_sb = persist.tile([P, 2, 64], f8e5)
            onesb_sb = persist.tile([P, 64], f16)
            dcmp_sb = persist.tile([P, 1], f32)
            if USE_FP8_PROJ:
                xq_sb = persist.tile([P, 4, 2, S], f8e4)
                wqk_sb = persist.tile([P, 4, 2, 512], f8e4)
                wv_sb = persist.tile([P, 4, 2, 256], f8e4)
            else:
                xq_sb = persist.tile([P, 8, S], bf16)
                wqk_sb = persist.tile([P, 8, 512], bf16)
                wv_sb = persist.tile([P, 8, 256], bf16)
            nc.sync.dma_start(out=bqk_sb[:], in_=bqk_d[:])
            nc.sync.dma_start(out=bvr_sb[:], in_=bvr_d[:])
            nc.sync.dma_start(out=mkb_sb[:], in_=mkb_d[:])
            nc.sync.dma_start(out=ones_sb[:], in_=ones_d[:])
            nc.sync.dma_start(out=onesb_sb[:], in_=onesb_d[:])
            nc.sync.dma_start(out=dcmp_sb[:], in_=dcmp_d[:])
            if USE_FP8_PROJ:
                for t in range(4):
                    nc.sync.dma_start(out=wqk_sb[:, t], in_=wqk_d[:, t])
                    nc.sync.dma_start(out=xq_sb[:, t], in_=xq_d[:, t])
                    nc.sync.dma_start(out=wv_sb[:, t], in_=wv_d[:, t])
            else:
                for t in range(8):
                    nc.sync.dma_start(out=wqk_sb[:, t], in_=wqk_d[:, t])
                    nc.sync.dma_start(out=xq_sb[:, t], in_=xq_d[:, t])
                    nc.sync.dma_start(out=wv_sb[:, t], in_=wv_d[:, t])

            with tc.tile_pool(name="estr", bufs=12) as ep, tc.tile_pool(
                name="astr", bufs=6
            ) as app, tc.tile_pool(name="ctp", bufs=6) as ctp, tc.tile_pool(
                name="small", bufs=1
            ) as smallp, tc.tile_pool(
                name="mm1ps", bufs=2, space="PSUM"
            ) as mm1p, tc.tile_pool(
                name="ops", bufs=1, space="PSUM"
            ) as pop, tc.tile_pool(
                name="rsps", bufs=1, space="PSUM"
            ) as prsp:

                def proj_qk(col, qb):
                    ps = mm1p.tile([P, S // 2], f32, tag="scT", name="pj")
                    if USE_FP8_PROJ:
                        for t in range(4):
                            nc.tensor.matmul(
                                ps[:, 0:512],
                                lhsT=wqk_sb[:, t, :, col * P : (col + 1) * P],
                                rhs=xq_sb[:, t, :, qb * 512 : (qb + 1) * 512],
                                perf_mode=DR,
                                start=(t == 0),
                                stop=(t == 3),
                            )
                    else:
                        for t in range(8):
                            nc.tensor.matmul(
                                ps[:, 0:512],
                                lhsT=wqk_sb[:, t, col * P : (col + 1) * P],
                                rhs=xq_sb[:, t, qb * 512 : (qb + 1) * 512],
                                start=(t == 0),
                                stop=(t == 7),
                            )
                    nc.vector.tensor_scalar_add(
                        out=qk_all[:, col, qb * 512 : (qb + 1) * 512],
                        in0=ps[:, 0:512],
                        scalar1=bqk_sb[:, col : col + 1],
                    )

                def proj_v(kt):
                    ps = mm1p.tile([P, S // 2], f32, tag="scT", name="pv")
                    if USE_FP8_PROJ:
                        for t in range(4):
                            nc.tensor.matmul(
                                ps[:, 0:256],
                                lhsT=xq_sb[:, t, :, kt * P : (kt + 1) * P],
                                rhs=wv_sb[:, t],
                                perf_mode=DR,
                                start=(t == 0),
                                stop=(t == 3),
                            )
                    else:
                        for t in range(8):
                            nc.tensor.matmul(
                                ps[:, 0:256],
                                lhsT=xq_sb[:, t, kt * P : (kt + 1) * P],
                                rhs=wv_sb[:, t],
                                start=(t == 0),
                                stop=(t == 7),
                            )
                    nc.vector.tensor_tensor(
                        v_sb[:, kt, :], ps[:, 0:256], bvr_sb[:], Alu.add
                    )

                def passA_strip(qh, pr, ks, eA, eB):
                    # scores^T [k=128, q=1024] per head into a 2-bank tile
                    pssA = mm1p.tile([P, S // 2], f32, tag="scT", name="pssA")
                    pssB = mm1p.tile([P, S // 2], f32, tag="scT", name="pssB")
                    for q2 in range(2):
                        qs = slice(qh * QH + q2 * 512, qh * QH + (q2 + 1) * 512)
                        os_ = slice(q2 * 512, (q2 + 1) * 512)
                        nc.tensor.matmul(
                            pssA[:, os_],
                            lhsT=qk_all[0:64, 2 * pr + 1, ks * P : (ks + 1) * P],
                            rhs=qk_all[0:64, 2 * pr, qs],
                            tile_position=(0, 0),
                            start=True,
                            stop=True,
                        )
                        nc.tensor.matmul(
                            pssB[:, os_],
                            lhsT=qk_all[64:128, 2 * pr + 1, ks * P : (ks + 1) * P],
                            rhs=qk_all[64:128, 2 * pr, qs],
                            tile_position=(64, 0),
                            start=True,
                            stop=True,
                        )
                    par = ks % 2
                    nc.scalar.activation(
                        eA[:, par, :],
                        pssA[:],
                        Act.Exp,
                        scale=EXPSCALE,
                        bias=mkb_sb[:, ks : ks + 1],
                    )
                    nc.scalar.activation(
                        eB[:, par, :],
                        pssB[:],
                        Act.Exp,
                        scale=EXPSCALE,
                        bias=mkb_sb[:, ks : ks + 1],
                    )

                def passB_strip(qh, pr, ks, eA, eB, po, prs):
                    h0, h1 = 2 * pr, 2 * pr + 1
                    par = ks % 2
                    ct = ctp.tile([P, QH], f16, tag="ct")
                    nc.sync.dma_start(
                        out=ct[:],
                        in_=ct_d[ks * P : (ks + 1) * P, qh * QH : (qh + 1) * QH],
                    )
                    aA = app.tile([P, QH], f16, tag="a")
                    aB = app.tile([P, QH], f16, tag="a")
                    nc.vector.tensor_tensor(aA[:], eA[:, par, :], ct[:], Alu.mult)
                    nc.vector.tensor_tensor(aB[:], eB[:, par, :], ct[:], Alu.mult)
                    st, sp = (ks == 0), (ks == NK - 1)
                    for q2 in range(2):
                        os_ = slice(q2 * 512, (q2 + 1) * 512)
                        nc.tensor.matmul(
                            po[0:64, os_],
                            lhsT=v_sb[:, ks, h0 * 64 : (h0 + 1) * 64],
                            rhs=aA[:, os_],
                            tile_position=(0, 0),
                            start=st,
                            stop=sp,
                        )
                        nc.tensor.matmul(
                            po[64:128, os_],
                            lhsT=v_sb[:, ks, h1 * 64 : (h1 + 1) * 64],
                            rhs=aB[:, os_],
                            tile_position=(0, 64),
                            start=st,
                            stop=sp,
                        )
                    # denominators: regular f16 ones matmuls per strip
                    for q2 in range(2):
                        os_ = slice(q2 * 512, (q2 + 1) * 512)
                        nc.tensor.matmul(
                            prs[0:64, os_],
                            lhsT=onesb_sb[:],
                            rhs=eA[:, par, os_],
                            tile_position=(0, 0),
                            start=st,
                            stop=sp,
                        )
                        nc.tensor.matmul(
                            prs[64:128, os_],
                            lhsT=onesb_sb[:],
                            rhs=eB[:, par, os_],
                            tile_position=(0, 64),
                            start=st,
                            stop=sp,
                        )

                def finishB(qh, pr, po, prs):
                    rcs = smallp.tile([P, QH], f32, tag="rcs")
                    scr = smallp.tile([P, QH], f32, tag="scr")
                    nc.vector.reciprocal_approx_accurate(rcs[:], prs[:], scr[:])
                    ob = smallp.tile([P, QH], f32, tag="ob")
                    nc.vector.scalar_tensor_tensor(
                        ob[:], po[:], dcmp_sb[:, 0:1], rcs[:], Alu.mult, Alu.mult
                    )
                    nc.sync.dma_start(
                        out=out_d[pr * P : (pr + 1) * P, qh * QH : (qh + 1) * QH],
                        in_=ob[:],
                    )

                phases = [(qh, pr) for qh in range(2) for pr in range(2)]

                # projection schedule: (col, qb) for qk / ('v', kt) for v,
                # ordered by need-by iteration; drained ~1/iteration.
                projq = (
                    [("qk", 1, 1), ("v", 0), ("v", 1), ("qk", 1, 2)]
                    + [("v", 2), ("v", 3), ("qk", 1, 3), ("v", 4)]
                    + [("v", 5), ("v", 6), ("v", 7), ("v", 8), ("v", 9)]
                    + [("qk", 2, 0), ("qk", 2, 1), ("qk", 3, 0), ("v", 10)]
                    + [("v", 11), ("qk", 3, 1), ("v", 12), ("v", 13)]
                    + [("qk", 3, 2), ("v", 14), ("v", 15), ("qk", 3, 3)]
                    + [("qk", 0, 2), ("qk", 0, 3), ("qk", 2, 2), ("qk", 2, 3)]
                )
                deadlines = {
                    ("qk", 1, 1): 4, ("qk", 1, 2): 8, ("qk", 1, 3): 12,
                    ("qk", 2, 0): 16, ("qk", 2, 1): 16, ("qk", 3, 0): 16,
                    ("qk", 3, 1): 20, ("qk", 3, 2): 24, ("qk", 3, 3): 28,
                    ("qk", 0, 2): 32, ("qk", 0, 3): 32,
                    ("qk", 2, 2): 48, ("qk", 2, 3): 48,
                }
                for kt in range(NK):
                    deadlines[("v", kt)] = kt + BOFF

                def emit_proj(item):
                    if item[0] == "qk":
                        proj_qk(item[1], item[2])
                    else:
                        proj_v(item[1])

                # prologue: Q(pair0, qh0) + first K(pair0) block
                proj_qk(0, 0)
                proj_qk(0, 1)
                proj_qk(1, 0)

                es_all = {}
                bstate = {}
                pptr = [0]
                NITER = 4 * NK + BOFF
                for g in range(NITER):
                    ph, ks = g // NK, g % NK
                    if g < 4 * NK:
                        qh, pr = phases[ph]
                        if ks % 2 == 0:
                            eA = ep.tile([P, 2, QH], f16, tag="e", name="eA")
                            eB = ep.tile([P, 2, QH], f16, tag="e", name="eB")
                            es_all.setdefault(ph, []).append((eA, eB))
                        eA, eB = es_all[ph][ks // 2]
                        passA_strip(qh, pr, ks, eA, eB)
                    # drain projection queue (deadline-forced, else 1/iter)
                    emitted = 0
                    while pptr[0] < len(projq) and (
                        deadlines[projq[pptr[0]]] <= g + 2 or emitted == 0
                    ):
                        emit_proj(projq[pptr[0]])
                        pptr[0] += 1
                        emitted += 1
                        if emitted >= 2 and not (
                            pptr[0] < len(projq)
                            and deadlines[projq[pptr[0]]] <= g + 2
                        ):
                            break
                    bg = g - BOFF
                    if bg >= 0:
                        bph, bks = bg // NK, bg % NK
                        bqh, bpr = phases[bph]
                        if bks == 0:
                            b_po = pop.tile([P, QH], f32, tag="po", name="po")
                            b_prs = prsp.tile([P, QH], f32, tag="prs", name="prs")
                            bstate[bph] = (b_po, b_prs)
                        b_po, b_prs = bstate[bph]
                        eA, eB = es_all[bph][bks // 2]
                        passB_strip(bqh, bpr, bks, eA, eB, b_po, b_prs)
                        if bks == NK - 1:
                            finishB(bqh, bpr, b_po, b_prs)
                            del bstate[bph]
                            del es_all[bph]

    nc.finalize()
    return nc


def _get_program():
    if "nc" not in _prog_cache:
        _prog_cache["nc"] = _build_program()
    return _prog_cache["nc"]


def kernel(x, attention_mask, C_prior, Wq, bq, Wk, bk, Wv, bv):
    from concourse.bass_utils import run_bass_kernel_spmd

    x = np.asarray(x, dtype=np.float32)
    attention_mask = np.asarray(attention_mask)
    C_prior = np.asarray(C_prior, dtype=np.float32)
    Wq = np.asarray(Wq, dtype=np.float32)
    Wk = np.asarray(Wk, dtype=np.float32)
    Wv = np.asarray(Wv, dtype=np.float32)
    bq = np.asarray(bq, dtype=np.float32)
    bk = np.asarray(bk, dtype=np.float32)
    bv = np.asarray(bv, dtype=np.float32)
    bf = ml_dtypes.bfloat16
    f8e4 = ml_dtypes.float8_e4m3fn
    f8e5 = ml_dtypes.float8_e5m2

    WqT, WkT, WvT = Wq.T, Wk.T, Wv.T  # [in D, out D]
    maskf = attention_mask.astype(np.float32)  # [B, S]

    def dr_pack(a):
        # [D, M] -> [128, 4, 2, M] with d = t*256 + i*128 + p
        Dd, M = a.shape
        return np.ascontiguousarray(
            a.reshape(4, 2, P, M).transpose(2, 0, 1, 3)
        )

    in_maps = []
    for c in range(NCORES):
        b, hg = c // 4, c % 4
        heads = [4 * hg + i for i in range(HEADS_PER_CORE)]

        wqk = np.empty((D, 512), np.float32)
        bqk = np.zeros((P, 4), np.float32)
        for pr in range(2):
            h0, h1 = heads[2 * pr], heads[2 * pr + 1]
            wqk[:, (2 * pr) * P : (2 * pr) * P + 64] = WqT[:, h0 * 64 : h0 * 64 + 64]
            wqk[:, (2 * pr) * P + 64 : (2 * pr + 1) * P] = WqT[
                :, h1 * 64 : h1 * 64 + 64
            ]
            wqk[:, (2 * pr + 1) * P : (2 * pr + 1) * P + 64] = WkT[
                :, h0 * 64 : h0 * 64 + 64
            ]
            wqk[:, (2 * pr + 1) * P + 64 : (2 * pr + 2) * P] = WkT[
                :, h1 * 64 : h1 * 64 + 64
            ]
            bqk[0:64, 2 * pr] = bq[h0 * 64 : h0 * 64 + 64]
            bqk[64:128, 2 * pr] = bq[h1 * 64 : h1 * 64 + 64]
            bqk[0:64, 2 * pr + 1] = bk[h0 * 64 : h0 * 64 + 64]
            bqk[64:128, 2 * pr + 1] = bk[h1 * 64 : h1 * 64 + 64]

        wv = np.ascontiguousarray(WvT[:, heads[0] * 64 : (heads[-1] + 1) * 64])
        bvr_v = bv[heads[0] * 64 : (heads[-1] + 1) * 64]

        xT = np.ascontiguousarray(x[b].T)  # [D, S]
        if USE_FP8_PROJ:
            xq = dr_pack(xT).astype(f8e4)
            wqk_in = dr_pack(wqk * W8).astype(f8e4)
            wv_in = dr_pack(wv * W8).astype(f8e4)
            bqk_in = bqk * W8
            bvr_in = np.ascontiguousarray(
                np.broadcast_to(bvr_v[None, :] * W8, (P, 256))
            ).astype(np.float32)
        else:
            xq = xT.reshape(8, P, S).transpose(1, 0, 2).astype(bf)
            xq = np.ascontiguousarray(xq)
            wqk_in = np.ascontiguousarray(
                wqk.reshape(8, P, 512).transpose(1, 0, 2)
            ).astype(bf)
            wv_in = np.ascontiguousarray(
                wv.reshape(8, P, 256).transpose(1, 0, 2)
            ).astype(bf)
            bqk_in = bqk
            bvr_in = np.ascontiguousarray(
                np.broadcast_to(bvr_v[None, :], (P, 256))
            ).astype(np.float32)

        m = maskf[b]  # [S]
        ct = (C_prior[b].T * m[:, None]).astype(np.float16)  # [S(k), S(q)]
        # exp bias per (k-partition, strip): round-to-nearest offset + mask
        mkb = np.full((P, NK), ROUND_BIAS, np.float32)
        mkb += np.where(m.reshape(NK, P).T > 0, 0.0, -1e9).astype(np.float32)
        vs = W8 if USE_FP8_PROJ else 1.0
        ones3 = np.full((P, 2, 64), vs, f8e5)
        onesb = np.full((P, 64), vs, np.float16)
        dcmp = np.ones((P, 1), np.float32)

        in_maps.append(
            {
                "xq": xq,
                "wqk": wqk_in,
                "wv": wv_in,
                "bqk": bqk_in,
                "bvr": bvr_in,
                "ct": ct,
                "mkb": mkb,
                "ones3": ones3,
                "onesb": onesb,
                "dcmp": dcmp,
            }
        )

    nc = _get_program()
    trace = bool(int(os.environ.get("BASS_KERNEL_TRACE", "0")))
    res = run_bass_kernel_spmd(nc, in_maps, list(range(NCORES)), trace=trace)
    if trace:
        print(f"HW exec time: {res.exec_time_ns} ns")
        _prog_cache["last_exec_time_ns"] = res.exec_time_ns
        _prog_cache["last_trace"] = res.instructions_and_trace

    out = np.empty((B, S, D), np.float32)
    for c in range(NCORES):
        b, hg = c // 4, c % 4
        co = res.results[c]["out"]  # [256, S]
        for i in range(HEADS_PER_CORE):
            h = 4 * hg + i
            out[b, :, h * 64 : (h + 1) * 64] = co[i * 64 : (i + 1) * 64, :].T
    return out


# revision 17
# speedup vs baseline: 1.0686x; 1.0142x over previous
"""Trainium2 Bass kernel for BertSelfAttention with C_prior multiply.

Reference (per batch b):
  q/k/v = x @ W{q,k,v}.T + b{q,k,v}            -> [S, D], H=16 heads of W=64
  scores = q k^T / sqrt(W); mask; softmax over k
  attn = softmax(scores) * C_prior[b]
  out = attn @ v                               -> [B, S, D]

Shapes: B=2, S=2048, D=1024, H=16, W=64.
Sharding: 8 cores; core c owns batch b=c//4 and 4 heads (hg=c%4).

Measured HW model driving the design: PE matmul time ~ max(PSUM f32
writes / 265G/s, rhs cols / 3.9G/s); Act = 128 lanes @1.2GHz; DVE gets
2x on 16-bit SBUF ops; Pool engine (nc.gpsimd) is a second 1.2GHz
vector engine.

Design vs the naive version:
  - Projections in fp8e4m3 DoubleRow: contraction 256 per step -> 4
    PSUM accumulation passes instead of 8 (proj PE time halves).
    Weights are pre-scaled x8 on host (so they sit in e4m3 normal
    range); compensated via the exp scale (q,k) and the broadcast
    constant (v).
  - scores^T [k,q] per (strip, head) land in a [128,1024] PSUM tile
    (2 banks); ONE exp per (strip, head) -> halves Act instruction count.
  - e is stored fp16. The softmax denominator is computed by a
    DoubleRow fp8e5m2 matmul whose rhs is the strided high-byte view
    of the fp16 e tile (fp16 high byte == e5m2 truncation). exp is
    biased by ln(1.0625) (half an m2 ulp) so truncation becomes
    round-to-nearest; the 1.0625 cancels between numerator and
    denominator. Thin [1,512] outputs: denominator streams 2 strips
    per call and writes almost nothing.
  - 1/denom via thin reciprocal (DVE) then broadcast up to 128
    partitions with a tiny f32r matmul whose host constant also holds
    the 1/8 fp8-V compensation.
  - mask folded into the per-partition exp bias; attn*C multiplies
    split between DVE and Pool; projection bias-adds (PSUM->SBUF
    moves) on Pool.
"""

import math
import os

import numpy as np
import ml_dtypes

B, S, D, H, W = 2, 2048, 1024, 16, 64
NCORES = 8
HEADS_PER_CORE = 4
P = 128
QH = S // 2  # q processed in two halves of 1024
NK = S // P  # 16 k-strips
BOFF = 10  # pass B trails pass A by this many strips

USE_FP8_PROJ = False
ROUND_BIAS = 0.0  # no pre-scale: truncation bias is scale-invariant
E5_TRUNC_RATIO = 0.91578  # E[e5m2_trunc(x)/x] for exp(N(0,1)) mantissas
W8 = 8.0  # host pre-scale on all projection weights (fp8 range)

_prog_cache = {}


def _build_program():
    import concourse.mybir as mybir
    import concourse.tile as tile
    from concourse import bacc

    dt = mybir.dt
    f32, bf16, f16 = dt.float32, dt.bfloat16, dt.float16
    f8e4, f8e5, f32r = dt.float8e4, dt.float8e5, dt.float32r
    Alu = mybir.AluOpType
    Act = mybir.ActivationFunctionType
    DR = mybir.MatmulPerfMode.DoubleRow

    nc = bacc.Bacc("TRN2", target_bir_lowering=False)

    if USE_FP8_PROJ:
        # x/W laid out for DoubleRow: contraction d = t*256 + i*128 + p
        xq_d = nc.declare_dram_parameter("xq", [P, 4, 2, S], f8e4, isOutput=False)
        wqk_d = nc.declare_dram_parameter("wqk", [P, 4, 2, 512], f8e4, isOutput=False)
        wv_d = nc.declare_dram_parameter("wv", [P, 4, 2, 256], f8e4, isOutput=False)
    else:
        xq_d = nc.declare_dram_parameter("xq", [P, 8, S], bf16, isOutput=False)
        wqk_d = nc.declare_dram_parameter("wqk", [P, 8, 512], bf16, isOutput=False)
        wv_d = nc.declare_dram_parameter("wv", [P, 8, 256], bf16, isOutput=False)
    bqk_d = nc.declare_dram_parameter("bqk", [P, 4], f32, isOutput=False)
    bvr_d = nc.declare_dram_parameter("bvr", [P, 256], f32, isOutput=False)
    ct_d = nc.declare_dram_parameter("ct", [S, S], bf16, isOutput=False)
    mkb_d = nc.declare_dram_parameter("mkb", [P, NK], f32, isOutput=False)
    onesb_d = nc.declare_dram_parameter("onesb", [P, 64], bf16, isOutput=False)
    out_d = nc.declare_dram_parameter("out", [256, S], f32, isOutput=True)

    EXPSCALE = 0.125 / (W8 * W8) if USE_FP8_PROJ else 0.125

    with tile.TileContext(nc) as tc:
        with tc.tile_pool(name="persist", bufs=1) as persist:
            qk_all = persist.tile([P, 4, S], bf16)
            v_sb = persist.tile([P, NK, 256], bf16)
            bqk_sb = persist.tile([P, 4], f32)
            bvr_sb = persist.tile([P, 256], f32)
            mkb_sb = persist.tile([P, NK], f32)
            onesb_sb = persist.tile([P, 64], bf16)
            if USE_FP8_PROJ:
                xq_sb = persist.tile([P, 4, 2, S], f8e4)
                wqk_sb = persist.tile([P, 4, 2, 512], f8e4)
                wv_sb = persist.tile([P, 4, 2, 256], f8e4)
            else:
                xq_sb = persist.tile([P, 8, S], bf16)
                wqk_sb = persist.tile([P, 8, 512], bf16)
                wv_sb = persist.tile([P, 8, 256], bf16)
            nc.sync.dma_start(out=bqk_sb[:], in_=bqk_d[:])
            nc.sync.dma_start(out=bvr_sb[:], in_=bvr_d[:])
            nc.sync.dma_start(out=mkb_sb[:], in_=mkb_d[:])
            nc.sync.dma_start(out=onesb_sb[:], in_=onesb_d[:])
            for t in range(8):
                nc.sync.dma_start(out=wqk_sb[:, t], in_=wqk_d[:, t])
            for qb in range(4):
                for t in range(8):
                    nc.sync.dma_start(
                        out=xq_sb[:, t, qb * 512 : (qb + 1) * 512],
                        in_=xq_d[:, t, qb * 512 : (qb + 1) * 512],
                    )
            for t in range(8):
                nc.sync.dma_start(out=wv_sb[:, t], in_=wv_d[:, t])

            with tc.tile_pool(name="estr", bufs=16) as ep, tc.tile_pool(
                name="astr", bufs=6
            ) as app, tc.tile_pool(name="ctp", bufs=6) as ctp, tc.tile_pool(
                name="small", bufs=1
            ) as smallp, tc.tile_pool(
                name="mm1ps", bufs=2, space="PSUM"
            ) as mm1p, tc.tile_pool(
                name="ops", bufs=1, space="PSUM"
            ) as pop, tc.tile_pool(
                name="rsps", bufs=1, space="PSUM"
            ) as prsp:

                def proj_qk(col, qb):
                    ps = mm1p.tile([P, S // 2], f32, tag="scT", name="pj")
                    if USE_FP8_PROJ:
                        for t in range(4):
                            nc.tensor.matmul(
                                ps[:, 0:512],
                                lhsT=wqk_sb[:, t, :, col * P : (col + 1) * P],
                                rhs=xq_sb[:, t, :, qb * 512 : (qb + 1) * 512],
                                perf_mode=DR,
                                start=(t == 0),
                                stop=(t == 3),
                            )
                    else:
                        for t in range(8):
                            nc.tensor.matmul(
                                ps[:, 0:512],
                                lhsT=wqk_sb[:, t, col * P : (col + 1) * P],
                                rhs=xq_sb[:, t, qb * 512 : (qb + 1) * 512],
                                start=(t == 0),
                                stop=(t == 7),
                            )
                    nc.vector.tensor_scalar_add(
                        out=qk_all[:, col, qb * 512 : (qb + 1) * 512],
                        in0=ps[:, 0:512],
                        scalar1=bqk_sb[:, col : col + 1],
                    )

                def proj_v(kt):
                    ps = mm1p.tile([P, S // 2], f32, tag="scT", name="pv")
                    if USE_FP8_PROJ:
                        for t in range(4):
                            nc.tensor.matmul(
                                ps[:, 0:256],
                                lhsT=xq_sb[:, t, :, kt * P : (kt + 1) * P],
                                rhs=wv_sb[:, t],
                                perf_mode=DR,
                                start=(t == 0),
                                stop=(t == 3),
                            )
                    else:
                        for t in range(8):
                            nc.tensor.matmul(
                                ps[:, 0:256],
                                lhsT=xq_sb[:, t, kt * P : (kt + 1) * P],
                                rhs=wv_sb[:, t],
                                start=(t == 0),
                                stop=(t == 7),
                            )
                    nc.vector.tensor_tensor(
                        v_sb[:, kt, :], ps[:, 0:256], bvr_sb[:], Alu.add
                    )

                def passA_strip(qh, pr, ks, eA, eB):
                    # scores^T [k=128, q=1024] per head into a 2-bank tile
                    pssA = mm1p.tile([P, S // 2], f32, tag="scT", name="pssA")
                    pssB = mm1p.tile([P, S // 2], f32, tag="scT", name="pssB")
                    for q2 in range(2):
                        qs = slice(qh * QH + q2 * 512, qh * QH + (q2 + 1) * 512)
                        os_ = slice(q2 * 512, (q2 + 1) * 512)
                        nc.tensor.matmul(
                            pssA[:, os_],
                            lhsT=qk_all[0:64, 2 * pr + 1, ks * P : (ks + 1) * P],
                            rhs=qk_all[0:64, 2 * pr, qs],
                            tile_position=(0, 0),
                            start=True,
                            stop=True,
                        )
                        nc.tensor.matmul(
                            pssB[:, os_],
                            lhsT=qk_all[64:128, 2 * pr + 1, ks * P : (ks + 1) * P],
                            rhs=qk_all[64:128, 2 * pr, qs],
                            tile_position=(64, 0),
                            start=True,
                            stop=True,
                        )
                    par = ks % 2
                    nc.scalar.activation(
                        eA[:, par, :],
                        pssA[:],
                        Act.Exp,
                        scale=EXPSCALE,
                        bias=mkb_sb[:, ks : ks + 1],
                    )
                    nc.scalar.activation(
                        eB[:, par, :],
                        pssB[:],
                        Act.Exp,
                        scale=EXPSCALE,
                        bias=mkb_sb[:, ks : ks + 1],
                    )

                def passB_strip(qh, pr, ks, eA, eB, po, prs):
                    h0, h1 = 2 * pr, 2 * pr + 1
                    par = ks % 2
                    ct = ctp.tile([P, QH], bf16, tag="ct")
                    nc.sync.dma_start(
                        out=ct[:],
                        in_=ct_d[ks * P : (ks + 1) * P, qh * QH : (qh + 1) * QH],
                    )
                    aA = app.tile([P, QH], bf16, tag="a")
                    aB = app.tile([P, QH], bf16, tag="a")
                    nc.vector.tensor_tensor(aA[:], eA[:, par, :], ct[:], Alu.mult)
                    nc.vector.tensor_tensor(aB[:], eB[:, par, :], ct[:], Alu.mult)
                    st, sp = (ks == 0), (ks == NK - 1)
                    for q2 in range(2):
                        os_ = slice(q2 * 512, (q2 + 1) * 512)
                        nc.tensor.matmul(
                            po[0:64, os_],
                            lhsT=v_sb[:, ks, h0 * 64 : (h0 + 1) * 64],
                            rhs=aA[:, os_],
                            tile_position=(0, 0),
                            start=st,
                            stop=sp,
                        )
                        nc.tensor.matmul(
                            po[64:128, os_],
                            lhsT=v_sb[:, ks, h1 * 64 : (h1 + 1) * 64],
                            rhs=aB[:, os_],
                            tile_position=(0, 64),
                            start=st,
                            stop=sp,
                        )
                    # denominators: regular f16 ones matmuls per strip
                    for q2 in range(2):
                        os_ = slice(q2 * 512, (q2 + 1) * 512)
                        nc.tensor.matmul(
                            prs[0:64, os_],
                            lhsT=onesb_sb[:],
                            rhs=eA[:, par, os_],
                            tile_position=(0, 0),
                            start=st,
                            stop=sp,
                        )
                        nc.tensor.matmul(
                            prs[64:128, os_],
                            lhsT=onesb_sb[:],
                            rhs=eB[:, par, os_],
                            tile_position=(0, 64),
                            start=st,
                            stop=sp,
                        )

                def finishB(qh, pr, po, prs):
                    rcs = smallp.tile([P, QH], f32, tag="rcs")
                    scr = smallp.tile([P, QH], f32, tag="scr")
                    nc.vector.reciprocal_approx_accurate(rcs[:], prs[:], scr[:])
                    ob = smallp.tile([P, QH], f32, tag="ob")
                    nc.vector.tensor_tensor(ob[:], po[:], rcs[:], Alu.mult)
                    nc.sync.dma_start(
                        out=out_d[pr * P : (pr + 1) * P, qh * QH : (qh + 1) * QH],
                        in_=ob[:],
                    )

                phases = [(qh, pr) for qh in range(2) for pr in range(2)]

                # projection schedule: (col, qb) for qk / ('v', kt) for v,
                # ordered by need-by iteration; drained ~1/iteration.
                projq = (
                    [("qk", 1, 1), ("qk", 1, 2), ("v", 0), ("v", 1)]
                    + [("qk", 1, 3), ("v", 2), ("v", 3), ("v", 4), ("v", 5)]
                    + [("qk", 2, 0), ("qk", 2, 1), ("qk", 3, 0)]
                    + [("v", 6), ("v", 7), ("v", 8), ("v", 9), ("qk", 3, 1)]
                    + [("v", 10), ("v", 11), ("v", 12), ("qk", 3, 2), ("v", 13)]
                    + [("v", 14), ("v", 15), ("qk", 3, 3)]
                    + [("qk", 0, 2), ("qk", 0, 3), ("qk", 2, 2), ("qk", 2, 3)]
                )
                deadlines = {
                    ("qk", 1, 1): 4, ("qk", 1, 2): 8, ("qk", 1, 3): 12,
                    ("qk", 2, 0): 16, ("qk", 2, 1): 16, ("qk", 3, 0): 16,
                    ("qk", 3, 1): 20, ("qk", 3, 2): 24, ("qk", 3, 3): 28,
                    ("qk", 0, 2): 32, ("qk", 0, 3): 32,
                    ("qk", 2, 2): 48, ("qk", 2, 3): 48,
                }
                for kt in range(NK):
                    deadlines[("v", kt)] = kt + BOFF

                def emit_proj(item):
                    if item[0] == "qk":
                        proj_qk(item[1], item[2])
                    else:
                        proj_v(item[1])

                # prologue: Q(pair0, qh0) + first K(pair0) block
                proj_qk(0, 0)
                proj_qk(0, 1)
                proj_qk(1, 0)

                es_all = {}
                bstate = {}
                pptr = [0]
                NITER = 4 * NK + BOFF
                for g in range(NITER):
                    ph, ks = g // NK, g % NK
                    if g < 4 * NK:
                        qh, pr = phases[ph]
                        if ks % 2 == 0:
                            eA = ep.tile([P, 2, QH], bf16, tag="e", name="eA")
                            eB = ep.tile([P, 2, QH], bf16, tag="e", name="eB")
                            es_all.setdefault(ph, []).append((eA, eB))
                        eA, eB = es_all[ph][ks // 2]
                        passA_strip(qh, pr, ks, eA, eB)
                    # drain projection queue (deadline-forced, else 1/iter)
                    emitted = 0
                    while pptr[0] < len(projq) and (
                        deadlines[projq[pptr[0]]] <= g + 2 or emitted == 0
                    ):
                        emit_proj(projq[pptr[0]])
                        pptr[0] += 1
                        emitted += 1
                        if emitted >= 2 and not (
                            pptr[0] < len(projq)
                            and deadlines[projq[pptr[0]]] <= g + 2
                        ):
                            break
                    bg = g - BOFF
                    if bg >= 0:
                        bph, bks = bg // NK, bg % NK
                        bqh, bpr = phases[bph]
                        if bks == 0:
                            b_po = pop.tile([P, QH], f32, tag="po", name="po")
                            b_prs = prsp.tile([P, QH], f32, tag="prs", name="prs")
                            bstate[bph] = (b_po, b_prs)
                        b_po, b_prs = bstate[bph]
                        eA, eB = es_all[bph][bks // 2]
                        passB_strip(bqh, bpr, bks, eA, eB, b_po, b_prs)
                        if bks == NK - 1:
                            finishB(bqh, bpr, b_po, b_prs)
                            del bstate[bph]
                            del es_all[bph]

    nc.finalize()
    return nc


def _get_program():
    if "nc" not in _prog_cache:
        _prog_cache["nc"] = _build_program()
    return _prog_cache["nc"]


def kernel(x, attention_mask, C_prior, Wq, bq, Wk, bk, Wv, bv):
    from concourse.bass_utils import run_bass_kernel_spmd

    x = np.asarray(x, dtype=np.float32)
    attention_mask = np.asarray(attention_mask)
    C_prior = np.asarray(C_prior, dtype=np.float32)
    Wq = np.asarray(Wq, dtype=np.float32)
    Wk = np.asarray(Wk, dtype=np.float32)
    Wv = np.asarray(Wv, dtype=np.float32)
    bq = np.asarray(bq, dtype=np.float32)
    bk = np.asarray(bk, dtype=np.float32)
    bv = np.asarray(bv, dtype=np.float32)
    bf = ml_dtypes.bfloat16
    f8e4 = ml_dtypes.float8_e4m3fn
    f8e5 = ml_dtypes.float8_e5m2

    WqT, WkT, WvT = Wq.T, Wk.T, Wv.T  # [in D, out D]
    maskf = attention_mask.astype(np.float32)  # [B, S]

    def dr_pack(a):
        # [D, M] -> [128, 4, 2, M] with d = t*256 + i*128 + p
        Dd, M = a.shape
        return np.ascontiguousarray(
            a.reshape(4, 2, P, M).transpose(2, 0, 1, 3)
        )

    in_maps = []
    for c in range(NCORES):
        b, hg = c // 4, c % 4
        heads = [4 * hg + i for i in range(HEADS_PER_CORE)]

        wqk = np.empty((D, 512), np.float32)
        bqk = np.zeros((P, 4), np.float32)
        for pr in range(2):
            h0, h1 = heads[2 * pr], heads[2 * pr + 1]
            wqk[:, (2 * pr) * P : (2 * pr) * P + 64] = WqT[:, h0 * 64 : h0 * 64 + 64]
            wqk[:, (2 * pr) * P + 64 : (2 * pr + 1) * P] = WqT[
                :, h1 * 64 : h1 * 64 + 64
            ]
            wqk[:, (2 * pr + 1) * P : (2 * pr + 1) * P + 64] = WkT[
                :, h0 * 64 : h0 * 64 + 64
            ]
            wqk[:, (2 * pr + 1) * P + 64 : (2 * pr + 2) * P] = WkT[
                :, h1 * 64 : h1 * 64 + 64
            ]
            bqk[0:64, 2 * pr] = bq[h0 * 64 : h0 * 64 + 64]
            bqk[64:128, 2 * pr] = bq[h1 * 64 : h1 * 64 + 64]
            bqk[0:64, 2 * pr + 1] = bk[h0 * 64 : h0 * 64 + 64]
            bqk[64:128, 2 * pr + 1] = bk[h1 * 64 : h1 * 64 + 64]

        wv = np.ascontiguousarray(WvT[:, heads[0] * 64 : (heads[-1] + 1) * 64])
        bvr_v = bv[heads[0] * 64 : (heads[-1] + 1) * 64]

        xT = np.ascontiguousarray(x[b].T)  # [D, S]
        if USE_FP8_PROJ:
            xq = dr_pack(xT).astype(f8e4)
            wqk_in = dr_pack(wqk * W8).astype(f8e4)
            wv_in = dr_pack(wv * W8).astype(f8e4)
            bqk_in = bqk * W8
            bvr_in = np.ascontiguousarray(
                np.broadcast_to(bvr_v[None, :] * W8, (P, 256))
            ).astype(np.float32)
        else:
            xq = xT.reshape(8, P, S).transpose(1, 0, 2).astype(bf)
            xq = np.ascontiguousarray(xq)
            wqk_in = np.ascontiguousarray(
                wqk.reshape(8, P, 512).transpose(1, 0, 2)
            ).astype(bf)
            wv_in = np.ascontiguousarray(
                wv.reshape(8, P, 256).transpose(1, 0, 2)
            ).astype(bf)
            bqk_in = bqk
            bvr_in = np.ascontiguousarray(
                np.broadcast_to(bvr_v[None, :], (P, 256))
            ).astype(np.float32)

        m = maskf[b]  # [S]
        ct = (C_prior[b].T * m[:, None]).astype(bf)  # [S(k), S(q)]
        # exp bias per (k-partition, strip): round-to-nearest offset + mask
        mkb = np.full((P, NK), ROUND_BIAS, np.float32)
        mkb += np.where(m.reshape(NK, P).T > 0, 0.0, -1e9).astype(np.float32)
        vs = W8 if USE_FP8_PROJ else 1.0
        onesb = np.full((P, 64), vs, bf)

        in_maps.append(
            {
                "xq": xq,
                "wqk": wqk_in,
                "wv": wv_in,
                "bqk": bqk_in,
                "bvr": bvr_in,
                "ct": ct,
                "mkb": mkb,
                "onesb": onesb,
            }
        )

    nc = _get_program()
    trace = bool(int(os.environ.get("BASS_KERNEL_TRACE", "0")))
    res = run_bass_kernel_spmd(nc, in_maps, list(range(NCORES)), trace=trace)
    if trace:
        print(f"HW exec time: {res.exec_time_ns} ns")
        _prog_cache["last_exec_time_ns"] = res.exec_time_ns
        _prog_cache["last_trace"] = res.instructions_and_trace

    out = np.empty((B, S, D), np.float32)
    for c in range(NCORES):
        b, hg = c // 4, c % 4
        co = res.results[c]["out"]  # [256, S]
        for i in range(HEADS_PER_CORE):
            h = 4 * hg + i
            out[b, :, h * 64 : (h + 1) * 64] = co[i * 64 : (i + 1) * 64, :].T
    return out


# revision 18
# speedup vs baseline: 1.1044x; 1.0335x over previous
"""Trainium2 Bass kernel for BertSelfAttention with C_prior multiply.

Reference (per batch b):
  q/k/v = x @ W{q,k,v}.T + b{q,k,v}            -> [S, D], H=16 heads of W=64
  scores = q k^T / sqrt(W); mask; softmax over k
  attn = softmax(scores) * C_prior[b]
  out = attn @ v                               -> [B, S, D]

Shapes: B=2, S=2048, D=1024, H=16, W=64.
Sharding: 8 cores; core c owns batch b=c//4 and 4 heads (hg=c%4).

Measured HW model driving the design: PE matmul time ~ max(PSUM f32
writes / 265G/s, rhs cols / 3.9G/s); Act = 128 lanes @1.2GHz; DVE gets
2x on 16-bit SBUF ops; Pool engine (nc.gpsimd) is a second 1.2GHz
vector engine.

Design vs the naive version:
  - Projections in fp8e4m3 DoubleRow: contraction 256 per step -> 4
    PSUM accumulation passes instead of 8 (proj PE time halves).
    Weights are pre-scaled x8 on host (so they sit in e4m3 normal
    range); compensated via the exp scale (q,k) and the broadcast
    constant (v).
  - scores^T [k,q] per (strip, head) land in a [128,1024] PSUM tile
    (2 banks); ONE exp per (strip, head) -> halves Act instruction count.
  - e is stored fp16. The softmax denominator is computed by a
    DoubleRow fp8e5m2 matmul whose rhs is the strided high-byte view
    of the fp16 e tile (fp16 high byte == e5m2 truncation). exp is
    biased by ln(1.0625) (half an m2 ulp) so truncation becomes
    round-to-nearest; the 1.0625 cancels between numerator and
    denominator. Thin [1,512] outputs: denominator streams 2 strips
    per call and writes almost nothing.
  - 1/denom via thin reciprocal (DVE) then broadcast up to 128
    partitions with a tiny f32r matmul whose host constant also holds
    the 1/8 fp8-V compensation.
  - mask folded into the per-partition exp bias; attn*C multiplies
    split between DVE and Pool; projection bias-adds (PSUM->SBUF
    moves) on Pool.
"""

import math
import os

import numpy as np
import ml_dtypes

B, S, D, H, W = 2, 2048, 1024, 16, 64
NCORES = 8
HEADS_PER_CORE = 4
P = 128
QH = S // 2  # q processed in two halves of 1024
NK = S // P  # 16 k-strips
BOFF = 10  # pass B trails pass A by this many strips

USE_FP8_PROJ = False
ROUND_BIAS = 0.0  # no pre-scale: truncation bias is scale-invariant
E5_TRUNC_RATIO = 0.91578  # E[e5m2_trunc(x)/x] for exp(N(0,1)) mantissas
W8 = 8.0  # host pre-scale on all projection weights (fp8 range)

_prog_cache = {}


def _build_program():
    import concourse.mybir as mybir
    import concourse.tile as tile
    from concourse import bacc

    dt = mybir.dt
    f32, bf16, f16 = dt.float32, dt.bfloat16, dt.float16
    f8e4, f8e5, f32r = dt.float8e4, dt.float8e5, dt.float32r
    Alu = mybir.AluOpType
    Act = mybir.ActivationFunctionType
    DR = mybir.MatmulPerfMode.DoubleRow

    nc = bacc.Bacc("TRN2", target_bir_lowering=False)

    if USE_FP8_PROJ:
        # x/W laid out for DoubleRow: contraction d = t*256 + i*128 + p
        xq_d = nc.declare_dram_parameter("xq", [P, 4, 2, S], f8e4, isOutput=False)
        wqk_d = nc.declare_dram_parameter("wqk", [P, 4, 2, 512], f8e4, isOutput=False)
        wv_d = nc.declare_dram_parameter("wv", [P, 4, 2, 256], f8e4, isOutput=False)
    else:
        xq_d = nc.declare_dram_parameter("xq", [P, 8, S], bf16, isOutput=False)
        wqk_d = nc.declare_dram_parameter("wqk", [P, 8, 512], bf16, isOutput=False)
        wv_d = nc.declare_dram_parameter("wv", [P, 8, 256], bf16, isOutput=False)
    bqk_d = nc.declare_dram_parameter("bqk", [P, 4], f32, isOutput=False)
    bvr_d = nc.declare_dram_parameter("bvr", [P, 256], f32, isOutput=False)
    ct_d = nc.declare_dram_parameter("ct", [S, S], bf16, isOutput=False)
    mkb_d = nc.declare_dram_parameter("mkb", [P, NK], f32, isOutput=False)
    onesb_d = nc.declare_dram_parameter("onesb", [P, 64], bf16, isOutput=False)
    out_d = nc.declare_dram_parameter("out", [256, S], f32, isOutput=True)

    EXPSCALE = 0.125 / (W8 * W8) if USE_FP8_PROJ else 0.125

    with tile.TileContext(nc) as tc:
        with tc.tile_pool(name="persist", bufs=1) as persist:
            qk_all = persist.tile([P, 4, S], bf16)
            v_sb = persist.tile([P, NK, 256], bf16)
            bqk_sb = persist.tile([P, 4], f32)
            bvr_sb = persist.tile([P, 256], f32)
            mkb_sb = persist.tile([P, NK], f32)
            onesb_sb = persist.tile([P, 64], bf16)
            if USE_FP8_PROJ:
                xq_sb = persist.tile([P, 4, 2, S], f8e4)
                wqk_sb = persist.tile([P, 4, 2, 512], f8e4)
                wv_sb = persist.tile([P, 4, 2, 256], f8e4)
            else:
                xq_sb = persist.tile([P, 8, S], bf16)
                wqk_sb = persist.tile([P, 8, 512], bf16)
                wv_sb = persist.tile([P, 8, 256], bf16)
            nc.sync.dma_start(out=bqk_sb[:], in_=bqk_d[:])
            nc.sync.dma_start(out=bvr_sb[:], in_=bvr_d[:])
            nc.sync.dma_start(out=mkb_sb[:], in_=mkb_d[:])
            nc.sync.dma_start(out=onesb_sb[:], in_=onesb_d[:])
            nc.sync.dma_start(out=wqk_sb[:, 0], in_=wqk_d[:, 0])
            for t in range(8):
                nc.sync.dma_start(
                    out=xq_sb[:, t, 0:512], in_=xq_d[:, t, 0:512]
                )
            for t in range(1, 8):
                nc.sync.dma_start(out=wqk_sb[:, t], in_=wqk_d[:, t])
            for qb in range(1, 4):
                for t in range(8):
                    nc.sync.dma_start(
                        out=xq_sb[:, t, qb * 512 : (qb + 1) * 512],
                        in_=xq_d[:, t, qb * 512 : (qb + 1) * 512],
                    )
            for t in range(8):
                nc.sync.dma_start(out=wv_sb[:, t], in_=wv_d[:, t])

            with tc.tile_pool(name="estr", bufs=16) as ep, tc.tile_pool(
                name="astr", bufs=6
            ) as app, tc.tile_pool(name="ctp", bufs=6) as ctp, tc.tile_pool(
                name="small", bufs=1
            ) as smallp, tc.tile_pool(
                name="mm1ps", bufs=4, space="PSUM"
            ) as mm1p, tc.tile_pool(
                name="ops", bufs=1, space="PSUM"
            ) as pop, tc.tile_pool(
                name="rsps", bufs=1, space="PSUM"
            ) as prsp:

                def proj_qk(col, qb):
                    ps = mm1p.tile([P, 512], f32, tag="scT", name="pj")
                    if USE_FP8_PROJ:
                        for t in range(4):
                            nc.tensor.matmul(
                                ps[:],
                                lhsT=wqk_sb[:, t, :, col * P : (col + 1) * P],
                                rhs=xq_sb[:, t, :, qb * 512 : (qb + 1) * 512],
                                perf_mode=DR,
                                start=(t == 0),
                                stop=(t == 3),
                            )
                    else:
                        for t in range(8):
                            nc.tensor.matmul(
                                ps[:],
                                lhsT=wqk_sb[:, t, col * P : (col + 1) * P],
                                rhs=xq_sb[:, t, qb * 512 : (qb + 1) * 512],
                                start=(t == 0),
                                stop=(t == 7),
                            )
                    nc.vector.tensor_scalar_add(
                        out=qk_all[:, col, qb * 512 : (qb + 1) * 512],
                        in0=ps[:],
                        scalar1=bqk_sb[:, col : col + 1],
                    )

                def proj_v(kt):
                    ps = mm1p.tile([P, 512], f32, tag="scT", name="pv")
                    if USE_FP8_PROJ:
                        for t in range(4):
                            nc.tensor.matmul(
                                ps[:, 0:256],
                                lhsT=xq_sb[:, t, :, kt * P : (kt + 1) * P],
                                rhs=wv_sb[:, t],
                                perf_mode=DR,
                                start=(t == 0),
                                stop=(t == 3),
                            )
                    else:
                        for t in range(8):
                            nc.tensor.matmul(
                                ps[:, 0:256],
                                lhsT=xq_sb[:, t, kt * P : (kt + 1) * P],
                                rhs=wv_sb[:, t],
                                start=(t == 0),
                                stop=(t == 7),
                            )
                    nc.vector.tensor_tensor(
                        v_sb[:, kt, :], ps[:, 0:256], bvr_sb[:], Alu.add
                    )

                def passA_strip(qh, pr, ks, eA, eB):
                    par = ks % 2
                    for q2 in range(2):
                        qs = slice(qh * QH + q2 * 512, qh * QH + (q2 + 1) * 512)
                        os_ = slice(q2 * 512, (q2 + 1) * 512)
                        psA = mm1p.tile([P, 512], f32, tag="scT", name="psA")
                        psB = mm1p.tile([P, 512], f32, tag="scT", name="psB")
                        nc.tensor.matmul(
                            psA[:],
                            lhsT=qk_all[0:64, 2 * pr + 1, ks * P : (ks + 1) * P],
                            rhs=qk_all[0:64, 2 * pr, qs],
                            tile_position=(0, 0),
                            start=True,
                            stop=True,
                        )
                        nc.tensor.matmul(
                            psB[:],
                            lhsT=qk_all[64:128, 2 * pr + 1, ks * P : (ks + 1) * P],
                            rhs=qk_all[64:128, 2 * pr, qs],
                            tile_position=(64, 0),
                            start=True,
                            stop=True,
                        )
                        nc.scalar.activation(
                            eA[:, par, os_],
                            psA[:],
                            Act.Exp,
                            scale=EXPSCALE,
                            bias=mkb_sb[:, ks : ks + 1],
                        )
                        nc.scalar.activation(
                            eB[:, par, os_],
                            psB[:],
                            Act.Exp,
                            scale=EXPSCALE,
                            bias=mkb_sb[:, ks : ks + 1],
                        )

                def passB_strip(qh, pr, ks, eA, eB, po, prs):
                    h0, h1 = 2 * pr, 2 * pr + 1
                    par = ks % 2
                    ct = ctp.tile([P, QH], bf16, tag="ct")
                    nc.sync.dma_start(
                        out=ct[:],
                        in_=ct_d[ks * P : (ks + 1) * P, qh * QH : (qh + 1) * QH],
                    )
                    aA = app.tile([P, QH], bf16, tag="a")
                    aB = app.tile([P, QH], bf16, tag="a")
                    nc.vector.tensor_tensor(aA[:], eA[:, par, :], ct[:], Alu.mult)
                    nc.vector.tensor_tensor(aB[:], eB[:, par, :], ct[:], Alu.mult)
                    st, sp = (ks == 0), (ks == NK - 1)
                    for q2 in range(2):
                        os_ = slice(q2 * 512, (q2 + 1) * 512)
                        nc.tensor.matmul(
                            po[0:64, os_],
                            lhsT=v_sb[:, ks, h0 * 64 : (h0 + 1) * 64],
                            rhs=aA[:, os_],
                            tile_position=(0, 0),
                            start=st,
                            stop=sp,
                        )
                        nc.tensor.matmul(
                            po[64:128, os_],
                            lhsT=v_sb[:, ks, h1 * 64 : (h1 + 1) * 64],
                            rhs=aB[:, os_],
                            tile_position=(0, 64),
                            start=st,
                            stop=sp,
                        )
                    # denominators: regular f16 ones matmuls per strip
                    for q2 in range(2):
                        os_ = slice(q2 * 512, (q2 + 1) * 512)
                        nc.tensor.matmul(
                            prs[0:64, os_],
                            lhsT=onesb_sb[:],
                            rhs=eA[:, par, os_],
                            tile_position=(0, 0),
                            start=st,
                            stop=sp,
                        )
                        nc.tensor.matmul(
                            prs[64:128, os_],
                            lhsT=onesb_sb[:],
                            rhs=eB[:, par, os_],
                            tile_position=(0, 64),
                            start=st,
                            stop=sp,
                        )

                def finishB(qh, pr, po, prs):
                    rcs = smallp.tile([P, QH], f32, tag="rcs")
                    scr = smallp.tile([P, QH], f32, tag="scr")
                    nc.vector.reciprocal_approx_accurate(rcs[:], prs[:], scr[:])
                    ob = smallp.tile([P, QH], f32, tag="ob")
                    for q2 in range(2):
                        os_ = slice(q2 * 512, (q2 + 1) * 512)
                        nc.vector.tensor_tensor(
                            ob[:, os_], po[:, os_], rcs[:, os_], Alu.mult
                        )
                        nc.sync.dma_start(
                            out=out_d[
                                pr * P : (pr + 1) * P,
                                qh * QH + q2 * 512 : qh * QH + (q2 + 1) * 512,
                            ],
                            in_=ob[:, os_],
                        )

                phases = [(qh, pr) for qh in range(2) for pr in range(2)]

                # projection schedule: (col, qb) for qk / ('v', kt) for v,
                # ordered by need-by iteration; drained ~1/iteration.
                projq = (
                    [("qk", 1, 1), ("qk", 1, 2), ("v", 0), ("v", 1)]
                    + [("qk", 1, 3), ("v", 2), ("v", 3), ("v", 4), ("v", 5)]
                    + [("qk", 2, 0), ("qk", 2, 1), ("qk", 3, 0)]
                    + [("v", 6), ("v", 7), ("v", 8), ("v", 9), ("qk", 3, 1)]
                    + [("v", 10), ("v", 11), ("v", 12), ("qk", 3, 2), ("v", 13)]
                    + [("v", 14), ("v", 15), ("qk", 3, 3)]
                    + [("qk", 0, 2), ("qk", 0, 3), ("qk", 2, 2), ("qk", 2, 3)]
                )
                deadlines = {
                    ("qk", 1, 1): 4, ("qk", 1, 2): 8, ("qk", 1, 3): 12,
                    ("qk", 2, 0): 16, ("qk", 2, 1): 16, ("qk", 3, 0): 16,
                    ("qk", 3, 1): 20, ("qk", 3, 2): 24, ("qk", 3, 3): 28,
                    ("qk", 0, 2): 32, ("qk", 0, 3): 32,
                    ("qk", 2, 2): 48, ("qk", 2, 3): 48,
                }
                for kt in range(NK):
                    deadlines[("v", kt)] = kt + BOFF

                def emit_proj(item):
                    if item[0] == "qk":
                        proj_qk(item[1], item[2])
                    else:
                        proj_v(item[1])

                # prologue: Q(pair0, qh0) + first K(pair0) block
                proj_qk(0, 0)
                proj_qk(0, 1)
                proj_qk(1, 0)

                es_all = {}
                bstate = {}
                pptr = [0]
                NITER = 4 * NK + BOFF
                for g in range(NITER):
                    ph, ks = g // NK, g % NK
                    if g < 4 * NK:
                        qh, pr = phases[ph]
                        if ks % 2 == 0:
                            eA = ep.tile([P, 2, QH], bf16, tag="e", name="eA")
                            eB = ep.tile([P, 2, QH], bf16, tag="e", name="eB")
                            es_all.setdefault(ph, []).append((eA, eB))
                        eA, eB = es_all[ph][ks // 2]
                        passA_strip(qh, pr, ks, eA, eB)
                    # drain projection queue (deadline-forced, else 1/iter)
                    emitted = 0
                    while pptr[0] < len(projq) and (
                        deadlines[projq[pptr[0]]] <= g + 2 or emitted == 0
                    ):
                        emit_proj(projq[pptr[0]])
                        pptr[0] += 1
                        emitted += 1
                        if emitted >= 2 and not (
                            pptr[0] < len(projq)
                            and deadlines[projq[pptr[0]]] <= g + 2
                        ):
                            break
                    bg = g - BOFF
                    if bg >= 0:
                        bph, bks = bg // NK, bg % NK
                        bqh, bpr = phases[bph]
                        if bks == 0:
                            b_po = pop.tile([P, QH], f32, tag="po", name="po")
                            b_prs = prsp.tile([P, QH], f32, tag="prs", name="prs")
                            bstate[bph] = (b_po, b_prs)
                        b_po, b_prs = bstate[bph]
                        eA, eB = es_all[bph][bks // 2]
                        passB_strip(bqh, bpr, bks, eA, eB, b_po, b_prs)
                        if bks == NK - 1:
                            finishB(bqh, bpr, b_po, b_prs)
                            del bstate[bph]
                            del es_all[bph]

    nc.finalize()
    return nc


def _get_program():
    if "nc" not in _prog_cache:
        _prog_cache["nc"] = _build_program()
    return _prog_cache["nc"]


def kernel(x, attention_mask, C_prior, Wq, bq, Wk, bk, Wv, bv):
    from concourse.bass_utils import run_bass_kernel_spmd

    x = np.asarray(x, dtype=np.float32)
    attention_mask = np.asarray(attention_mask)
    C_prior = np.asarray(C_prior, dtype=np.float32)
    Wq = np.asarray(Wq, dtype=np.float32)
    Wk = np.asarray(Wk, dtype=np.float32)
    Wv = np.asarray(Wv, dtype=np.float32)
    bq = np.asarray(bq, dtype=np.float32)
    bk = np.asarray(bk, dtype=np.float32)
    bv = np.asarray(bv, dtype=np.float32)
    bf = ml_dtypes.bfloat16
    f8e4 = ml_dtypes.float8_e4m3fn
    f8e5 = ml_dtypes.float8_e5m2

    WqT, WkT, WvT = Wq.T, Wk.T, Wv.T  # [in D, out D]
    maskf = attention_mask.astype(np.float32)  # [B, S]

    def dr_pack(a):
        # [D, M] -> [128, 4, 2, M] with d = t*256 + i*128 + p
        Dd, M = a.shape
        return np.ascontiguousarray(
            a.reshape(4, 2, P, M).transpose(2, 0, 1, 3)
        )

    in_maps = []
    for c in range(NCORES):
        b, hg = c // 4, c % 4
        heads = [4 * hg + i for i in range(HEADS_PER_CORE)]

        wqk = np.empty((D, 512), np.float32)
        bqk = np.zeros((P, 4), np.float32)
        for pr in range(2):
            h0, h1 = heads[2 * pr], heads[2 * pr + 1]
            wqk[:, (2 * pr) * P : (2 * pr) * P + 64] = WqT[:, h0 * 64 : h0 * 64 + 64]
            wqk[:, (2 * pr) * P + 64 : (2 * pr + 1) * P] = WqT[
                :, h1 * 64 : h1 * 64 + 64
            ]
            wqk[:, (2 * pr + 1) * P : (2 * pr + 1) * P + 64] = WkT[
                :, h0 * 64 : h0 * 64 + 64
            ]
            wqk[:, (2 * pr + 1) * P + 64 : (2 * pr + 2) * P] = WkT[
                :, h1 * 64 : h1 * 64 + 64
            ]
            bqk[0:64, 2 * pr] = bq[h0 * 64 : h0 * 64 + 64]
            bqk[64:128, 2 * pr] = bq[h1 * 64 : h1 * 64 + 64]
            bqk[0:64, 2 * pr + 1] = bk[h0 * 64 : h0 * 64 + 64]
            bqk[64:128, 2 * pr + 1] = bk[h1 * 64 : h1 * 64 + 64]

        wv = np.ascontiguousarray(WvT[:, heads[0] * 64 : (heads[-1] + 1) * 64])
        bvr_v = bv[heads[0] * 64 : (heads[-1] + 1) * 64]

        xT = np.ascontiguousarray(x[b].T)  # [D, S]
        if USE_FP8_PROJ:
            xq = dr_pack(xT).astype(f8e4)
            wqk_in = dr_pack(wqk * W8).astype(f8e4)
            wv_in = dr_pack(wv * W8).astype(f8e4)
            bqk_in = bqk * W8
            bvr_in = np.ascontiguousarray(
                np.broadcast_to(bvr_v[None, :] * W8, (P, 256))
            ).astype(np.float32)
        else:
            xq = xT.reshape(8, P, S).transpose(1, 0, 2).astype(bf)
            xq = np.ascontiguousarray(xq)
            wqk_in = np.ascontiguousarray(
                wqk.reshape(8, P, 512).transpose(1, 0, 2)
            ).astype(bf)
            wv_in = np.ascontiguousarray(
                wv.reshape(8, P, 256).transpose(1, 0, 2)
            ).astype(bf)
            bqk_in = bqk
            bvr_in = np.ascontiguousarray(
                np.broadcast_to(bvr_v[None, :], (P, 256))
            ).astype(np.float32)

        m = maskf[b]  # [S]
        ct = (C_prior[b].T * m[:, None]).astype(bf)  # [S(k), S(q)]
        # exp bias per (k-partition, strip): round-to-nearest offset + mask
        mkb = np.full((P, NK), ROUND_BIAS, np.float32)
        mkb += np.where(m.reshape(NK, P).T > 0, 0.0, -1e9).astype(np.float32)
        vs = W8 if USE_FP8_PROJ else 1.0
        onesb = np.full((P, 64), vs, bf)

        in_maps.append(
            {
                "xq": xq,
                "wqk": wqk_in,
                "wv": wv_in,
                "bqk": bqk_in,
                "bvr": bvr_in,
                "ct": ct,
                "mkb": mkb,
                "onesb": onesb,
            }
        )

    nc = _get_program()
    trace = bool(int(os.environ.get("BASS_KERNEL_TRACE", "0")))
    res = run_bass_kernel_spmd(nc, in_maps, list(range(NCORES)), trace=trace)
    if trace:
        print(f"HW exec time: {res.exec_time_ns} ns")
        _prog_cache["last_exec_time_ns"] = res.exec_time_ns
        _prog_cache["last_trace"] = res.instructions_and_trace

    out = np.empty((B, S, D), np.float32)
    for c in range(NCORES):
        b, hg = c // 4, c % 4
        co = res.results[c]["out"]  # [256, S]
        for i in range(HEADS_PER_CORE):
            h = 4 * hg + i
            out[b, :, h * 64 : (h + 1) * 64] = co[i * 64 : (i + 1) * 64, :].T
    return out


# revision 19
# speedup vs baseline: 1.1296x; 1.0228x over previous
"""Trainium2 Bass kernel for BertSelfAttention with C_prior multiply.

Reference (per batch b):
  q/k/v = x @ W{q,k,v}.T + b{q,k,v}            -> [S, D], H=16 heads of W=64
  scores = q k^T / sqrt(W); mask; softmax over k
  attn = softmax(scores) * C_prior[b]
  out = attn @ v                               -> [B, S, D]

Shapes: B=2, S=2048, D=1024, H=16, W=64.
Sharding: 8 cores; core c owns batch b=c//4 and 4 heads (hg=c%4).

Measured HW model driving the design: PE matmul time ~ max(PSUM f32
writes / 265G/s, rhs cols / 3.9G/s); Act = 128 lanes @1.2GHz; DVE gets
2x on 16-bit SBUF ops; Pool engine (nc.gpsimd) is a second 1.2GHz
vector engine.

Design vs the naive version:
  - Projections in fp8e4m3 DoubleRow: contraction 256 per step -> 4
    PSUM accumulation passes instead of 8 (proj PE time halves).
    Weights are pre-scaled x8 on host (so they sit in e4m3 normal
    range); compensated via the exp scale (q,k) and the broadcast
    constant (v).
  - scores^T [k,q] per (strip, head) land in a [128,1024] PSUM tile
    (2 banks); ONE exp per (strip, head) -> halves Act instruction count.
  - e is stored fp16. The softmax denominator is computed by a
    DoubleRow fp8e5m2 matmul whose rhs is the strided high-byte view
    of the fp16 e tile (fp16 high byte == e5m2 truncation). exp is
    biased by ln(1.0625) (half an m2 ulp) so truncation becomes
    round-to-nearest; the 1.0625 cancels between numerator and
    denominator. Thin [1,512] outputs: denominator streams 2 strips
    per call and writes almost nothing.
  - 1/denom via thin reciprocal (DVE) then broadcast up to 128
    partitions with a tiny f32r matmul whose host constant also holds
    the 1/8 fp8-V compensation.
  - mask folded into the per-partition exp bias; attn*C multiplies
    split between DVE and Pool; projection bias-adds (PSUM->SBUF
    moves) on Pool.
"""

import math
import os

import numpy as np
import ml_dtypes

B, S, D, H, W = 2, 2048, 1024, 16, 64
NCORES = 8
HEADS_PER_CORE = 4
P = 128
QH = S // 2  # q processed in two halves of 1024
NK = S // P  # 16 k-strips
BOFF = 10  # pass B trails pass A by this many strips

USE_FP8_PROJ = False
ROUND_BIAS = 0.0  # no pre-scale: truncation bias is scale-invariant
E5_TRUNC_RATIO = 0.91578  # E[e5m2_trunc(x)/x] for exp(N(0,1)) mantissas
W8 = 8.0  # host pre-scale on all projection weights (fp8 range)

_prog_cache = {}


def _build_program():
    import concourse.mybir as mybir
    import concourse.tile as tile
    from concourse import bacc

    dt = mybir.dt
    f32, bf16, f16 = dt.float32, dt.bfloat16, dt.float16
    f8e4, f8e5, f32r = dt.float8e4, dt.float8e5, dt.float32r
    Alu = mybir.AluOpType
    Act = mybir.ActivationFunctionType
    DR = mybir.MatmulPerfMode.DoubleRow

    nc = bacc.Bacc("TRN2", target_bir_lowering=False)

    if USE_FP8_PROJ:
        # x/W laid out for DoubleRow: contraction d = t*256 + i*128 + p
        xq_d = nc.declare_dram_parameter("xq", [P, 4, 2, S], f8e4, isOutput=False)
        wqk_d = nc.declare_dram_parameter("wqk", [P, 4, 2, 512], f8e4, isOutput=False)
        wv_d = nc.declare_dram_parameter("wv", [P, 4, 2, 256], f8e4, isOutput=False)
    else:
        xq_d = nc.declare_dram_parameter("xq", [P, 8, S], bf16, isOutput=False)
        wqk_d = nc.declare_dram_parameter("wqk", [P, 8, 512], bf16, isOutput=False)
        wv_d = nc.declare_dram_parameter("wv", [P, 8, 256], bf16, isOutput=False)
    bqk_d = nc.declare_dram_parameter("bqk", [P, 4], f32, isOutput=False)
    bvr_d = nc.declare_dram_parameter("bvr", [P, 256], f32, isOutput=False)
    ct_d = nc.declare_dram_parameter("ct", [S, S], bf16, isOutput=False)
    mkb_d = nc.declare_dram_parameter("mkb", [P, NK], f32, isOutput=False)
    onesb_d = nc.declare_dram_parameter("onesb", [P, 64], bf16, isOutput=False)
    out_d = nc.declare_dram_parameter("out", [256, S], f32, isOutput=True)

    EXPSCALE = 0.125 / (W8 * W8) if USE_FP8_PROJ else 0.125

    with tile.TileContext(nc) as tc:
        with tc.tile_pool(name="persist", bufs=1) as persist:
            qk_all = persist.tile([P, 4, S], bf16)
            v_sb = persist.tile([P, NK, 256], bf16)
            bqk_sb = persist.tile([P, 4], f32)
            bvr_sb = persist.tile([P, 256], f32)
            mkb_sb = persist.tile([P, NK], f32)
            onesb_sb = persist.tile([P, 64], bf16)
            if USE_FP8_PROJ:
                xq_sb = persist.tile([P, 4, 2, S], f8e4)
                wqk_sb = persist.tile([P, 4, 2, 512], f8e4)
                wv_sb = persist.tile([P, 4, 2, 256], f8e4)
            else:
                xq_sb = persist.tile([P, 8, S], bf16)
                wqk_sb = persist.tile([P, 8, 512], bf16)
                wv_sb = persist.tile([P, 8, 256], bf16)
            nc.sync.dma_start(out=bqk_sb[:], in_=bqk_d[:])
            nc.sync.dma_start(out=bvr_sb[:], in_=bvr_d[:])
            nc.sync.dma_start(out=mkb_sb[:], in_=mkb_d[:])
            nc.sync.dma_start(out=onesb_sb[:], in_=onesb_d[:])
            for t in range(8):
                nc.sync.dma_start(out=wqk_sb[:, t], in_=wqk_d[:, t])
                nc.sync.dma_start(
                    out=xq_sb[:, t, 0:512], in_=xq_d[:, t, 0:512]
                )
            for qb in range(1, 4):
                for t in range(8):
                    nc.sync.dma_start(
                        out=xq_sb[:, t, qb * 512 : (qb + 1) * 512],
                        in_=xq_d[:, t, qb * 512 : (qb + 1) * 512],
                    )
            for t in range(8):
                nc.sync.dma_start(out=wv_sb[:, t], in_=wv_d[:, t])

            with tc.tile_pool(name="estr", bufs=16) as ep, tc.tile_pool(
                name="astr", bufs=12
            ) as app, tc.tile_pool(name="ctp", bufs=8) as ctp, tc.tile_pool(
                name="small", bufs=1
            ) as smallp, tc.tile_pool(
                name="mm1ps", bufs=4, space="PSUM"
            ) as mm1p, tc.tile_pool(
                name="ops", bufs=1, space="PSUM"
            ) as pop, tc.tile_pool(
                name="rsps", bufs=1, space="PSUM"
            ) as prsp:

                def proj_qk(col, qb):
                    ps = mm1p.tile([P, 512], f32, tag="scT", name="pj")
                    if USE_FP8_PROJ:
                        for t in range(4):
                            nc.tensor.matmul(
                                ps[:],
                                lhsT=wqk_sb[:, t, :, col * P : (col + 1) * P],
                                rhs=xq_sb[:, t, :, qb * 512 : (qb + 1) * 512],
                                perf_mode=DR,
                                start=(t == 0),
                                stop=(t == 3),
                            )
                    else:
                        for t in range(8):
                            nc.tensor.matmul(
                                ps[:],
                                lhsT=wqk_sb[:, t, col * P : (col + 1) * P],
                                rhs=xq_sb[:, t, qb * 512 : (qb + 1) * 512],
                                start=(t == 0),
                                stop=(t == 7),
                            )
                    nc.vector.tensor_scalar_add(
                        out=qk_all[:, col, qb * 512 : (qb + 1) * 512],
                        in0=ps[:],
                        scalar1=bqk_sb[:, col : col + 1],
                    )

                def proj_v(kt):
                    ps = mm1p.tile([P, 512], f32, tag="scT", name="pv")
                    if USE_FP8_PROJ:
                        for t in range(4):
                            nc.tensor.matmul(
                                ps[:, 0:256],
                                lhsT=xq_sb[:, t, :, kt * P : (kt + 1) * P],
                                rhs=wv_sb[:, t],
                                perf_mode=DR,
                                start=(t == 0),
                                stop=(t == 3),
                            )
                    else:
                        for t in range(8):
                            nc.tensor.matmul(
                                ps[:, 0:256],
                                lhsT=xq_sb[:, t, kt * P : (kt + 1) * P],
                                rhs=wv_sb[:, t],
                                start=(t == 0),
                                stop=(t == 7),
                            )
                    nc.vector.tensor_tensor(
                        v_sb[:, kt, :], ps[:, 0:256], bvr_sb[:], Alu.add
                    )

                def passA_strip(qh, pr, ks, eA, eB):
                    par = ks % 2
                    for q2 in range(2):
                        qs = slice(qh * QH + q2 * 512, qh * QH + (q2 + 1) * 512)
                        os_ = slice(q2 * 512, (q2 + 1) * 512)
                        psA = mm1p.tile([P, 512], f32, tag="scT", name="psA")
                        psB = mm1p.tile([P, 512], f32, tag="scT", name="psB")
                        nc.tensor.matmul(
                            psA[:],
                            lhsT=qk_all[0:64, 2 * pr + 1, ks * P : (ks + 1) * P],
                            rhs=qk_all[0:64, 2 * pr, qs],
                            tile_position=(0, 0),
                            start=True,
                            stop=True,
                        )
                        nc.tensor.matmul(
                            psB[:],
                            lhsT=qk_all[64:128, 2 * pr + 1, ks * P : (ks + 1) * P],
                            rhs=qk_all[64:128, 2 * pr, qs],
                            tile_position=(64, 0),
                            start=True,
                            stop=True,
                        )
                        nc.scalar.activation(
                            eA[:, par, os_],
                            psA[:],
                            Act.Exp,
                            scale=EXPSCALE,
                            bias=mkb_sb[:, ks : ks + 1],
                        )
                        nc.scalar.activation(
                            eB[:, par, os_],
                            psB[:],
                            Act.Exp,
                            scale=EXPSCALE,
                            bias=mkb_sb[:, ks : ks + 1],
                        )

                def prepB_strip(qh, pr, ks, eA, eB):
                    par = ks % 2
                    ct = ctp.tile([P, QH], bf16, tag="ct")
                    nc.sync.dma_start(
                        out=ct[:],
                        in_=ct_d[ks * P : (ks + 1) * P, qh * QH : (qh + 1) * QH],
                    )
                    aA = app.tile([P, QH], bf16, tag="a")
                    aB = app.tile([P, QH], bf16, tag="a")
                    nc.vector.tensor_tensor(aA[:], eA[:, par, :], ct[:], Alu.mult)
                    nc.vector.tensor_tensor(aB[:], eB[:, par, :], ct[:], Alu.mult)
                    return aA, aB

                def passB_strip(qh, pr, ks, eA, eB, aA, aB, po, prs):
                    h0, h1 = 2 * pr, 2 * pr + 1
                    par = ks % 2
                    st, sp = (ks == 0), (ks == NK - 1)
                    for q2 in range(2):
                        os_ = slice(q2 * 512, (q2 + 1) * 512)
                        nc.tensor.matmul(
                            po[0:64, os_],
                            lhsT=v_sb[:, ks, h0 * 64 : (h0 + 1) * 64],
                            rhs=aA[:, os_],
                            tile_position=(0, 0),
                            start=st,
                            stop=sp,
                        )
                        nc.tensor.matmul(
                            po[64:128, os_],
                            lhsT=v_sb[:, ks, h1 * 64 : (h1 + 1) * 64],
                            rhs=aB[:, os_],
                            tile_position=(0, 64),
                            start=st,
                            stop=sp,
                        )
                    # denominators: regular f16 ones matmuls per strip
                    for q2 in range(2):
                        os_ = slice(q2 * 512, (q2 + 1) * 512)
                        nc.tensor.matmul(
                            prs[0:64, os_],
                            lhsT=onesb_sb[:],
                            rhs=eA[:, par, os_],
                            tile_position=(0, 0),
                            start=st,
                            stop=sp,
                        )
                        nc.tensor.matmul(
                            prs[64:128, os_],
                            lhsT=onesb_sb[:],
                            rhs=eB[:, par, os_],
                            tile_position=(0, 64),
                            start=st,
                            stop=sp,
                        )

                def finishB(qh, pr, po, prs):
                    rcs = smallp.tile([P, QH], f32, tag="rcs")
                    scr = smallp.tile([P, QH], f32, tag="scr")
                    nc.vector.reciprocal_approx_accurate(rcs[:], prs[:], scr[:])
                    ob = smallp.tile([P, QH], f32, tag="ob")
                    for q2 in range(2):
                        os_ = slice(q2 * 512, (q2 + 1) * 512)
                        nc.vector.tensor_tensor(
                            ob[:, os_], po[:, os_], rcs[:, os_], Alu.mult
                        )
                        nc.sync.dma_start(
                            out=out_d[
                                pr * P : (pr + 1) * P,
                                qh * QH + q2 * 512 : qh * QH + (q2 + 1) * 512,
                            ],
                            in_=ob[:, os_],
                        )

                phases = [(qh, pr) for qh in range(2) for pr in range(2)]

                # projection schedule: (col, qb) for qk / ('v', kt) for v,
                # ordered by need-by iteration; drained ~1/iteration.
                projq = (
                    [("qk", 1, 1), ("qk", 1, 2), ("v", 0), ("v", 1)]
                    + [("qk", 1, 3), ("v", 2), ("v", 3), ("v", 4), ("v", 5)]
                    + [("qk", 2, 0), ("qk", 2, 1), ("qk", 3, 0)]
                    + [("v", 6), ("v", 7), ("v", 8), ("v", 9), ("qk", 3, 1)]
                    + [("v", 10), ("v", 11), ("v", 12), ("qk", 3, 2), ("v", 13)]
                    + [("v", 14), ("v", 15), ("qk", 3, 3)]
                    + [("qk", 0, 2), ("qk", 0, 3), ("qk", 2, 2), ("qk", 2, 3)]
                )
                deadlines = {
                    ("qk", 1, 1): 4, ("qk", 1, 2): 8, ("qk", 1, 3): 12,
                    ("qk", 2, 0): 16, ("qk", 2, 1): 16, ("qk", 3, 0): 16,
                    ("qk", 3, 1): 20, ("qk", 3, 2): 24, ("qk", 3, 3): 28,
                    ("qk", 0, 2): 32, ("qk", 0, 3): 32,
                    ("qk", 2, 2): 48, ("qk", 2, 3): 48,
                }
                for kt in range(NK):
                    deadlines[("v", kt)] = kt + BOFF

                def emit_proj(item):
                    if item[0] == "qk":
                        proj_qk(item[1], item[2])
                    else:
                        proj_v(item[1])

                # prologue: Q(pair0, qh0) + first K(pair0) block
                proj_qk(0, 0)
                proj_qk(1, 0)
                proj_qk(0, 1)

                es_all = {}
                bstate = {}
                prepped = {}
                pptr = [0]
                PREP_AHEAD = 4
                NITER = 4 * NK + BOFF
                for g in range(NITER):
                    ph, ks = g // NK, g % NK
                    if g < 4 * NK:
                        qh, pr = phases[ph]
                        if ks % 2 == 0:
                            eA = ep.tile([P, 2, QH], bf16, tag="e", name="eA")
                            eB = ep.tile([P, 2, QH], bf16, tag="e", name="eB")
                            es_all.setdefault(ph, []).append((eA, eB))
                        eA, eB = es_all[ph][ks // 2]
                        passA_strip(qh, pr, ks, eA, eB)
                    # drain projection queue (deadline-forced, else 1/iter)
                    emitted = 0
                    while pptr[0] < len(projq) and (
                        deadlines[projq[pptr[0]]] <= g + 2 or emitted == 0
                    ):
                        emit_proj(projq[pptr[0]])
                        pptr[0] += 1
                        emitted += 1
                        if emitted >= 2 and not (
                            pptr[0] < len(projq)
                            and deadlines[projq[pptr[0]]] <= g + 2
                        ):
                            break
                    pg = g - (BOFF - PREP_AHEAD)
                    if 0 <= pg < 4 * NK:
                        pph, pks = pg // NK, pg % NK
                        pqh, ppr = phases[pph]
                        peA, peB = es_all[pph][pks // 2]
                        prepped[pg] = prepB_strip(pqh, ppr, pks, peA, peB)
                    bg = g - BOFF
                    if bg >= 0:
                        bph, bks = bg // NK, bg % NK
                        bqh, bpr = phases[bph]
                        if bks == 0:
                            b_po = pop.tile([P, QH], f32, tag="po", name="po")
                            b_prs = prsp.tile([P, QH], f32, tag="prs", name="prs")
                            bstate[bph] = (b_po, b_prs)
                        b_po, b_prs = bstate[bph]
                        eA, eB = es_all[bph][bks // 2]
                        aA, aB = prepped.pop(bg)
                        passB_strip(bqh, bpr, bks, eA, eB, aA, aB, b_po, b_prs)
                        if bks == NK - 1:
                            finishB(bqh, bpr, b_po, b_prs)
                            del bstate[bph]
                            del es_all[bph]

    nc.finalize()
    return nc


def _get_program():
    if "nc" not in _prog_cache:
        _prog_cache["nc"] = _build_program()
    return _prog_cache["nc"]


def kernel(x, attention_mask, C_prior, Wq, bq, Wk, bk, Wv, bv):
    from concourse.bass_utils import run_bass_kernel_spmd

    x = np.asarray(x, dtype=np.float32)
    attention_mask = np.asarray(attention_mask)
    C_prior = np.asarray(C_prior, dtype=np.float32)
    Wq = np.asarray(Wq, dtype=np.float32)
    Wk = np.asarray(Wk, dtype=np.float32)
    Wv = np.asarray(Wv, dtype=np.float32)
    bq = np.asarray(bq, dtype=np.float32)
    bk = np.asarray(bk, dtype=np.float32)
    bv = np.asarray(bv, dtype=np.float32)
    bf = ml_dtypes.bfloat16
    f8e4 = ml_dtypes.float8_e4m3fn
    f8e5 = ml_dtypes.float8_e5m2

    WqT, WkT, WvT = Wq.T, Wk.T, Wv.T  # [in D, out D]
    maskf = attention_mask.astype(np.float32)  # [B, S]

    def dr_pack(a):
        # [D, M] -> [128, 4, 2, M] with d = t*256 + i*128 + p
        Dd, M = a.shape
        return np.ascontiguousarray(
            a.reshape(4, 2, P, M).transpose(2, 0, 1, 3)
        )

    in_maps = []
    for c in range(NCORES):
        b, hg = c // 4, c % 4
        heads = [4 * hg + i for i in range(HEADS_PER_CORE)]

        wqk = np.empty((D, 512), np.float32)
        bqk = np.zeros((P, 4), np.float32)
        for pr in range(2):
            h0, h1 = heads[2 * pr], heads[2 * pr + 1]
            wqk[:, (2 * pr) * P : (2 * pr) * P + 64] = WqT[:, h0 * 64 : h0 * 64 + 64]
            wqk[:, (2 * pr) * P + 64 : (2 * pr + 1) * P] = WqT[
                :, h1 * 64 : h1 * 64 + 64
            ]
            wqk[:, (2 * pr + 1) * P : (2 * pr + 1) * P + 64] = WkT[
                :, h0 * 64 : h0 * 64 + 64
            ]
            wqk[:, (2 * pr + 1) * P + 64 : (2 * pr + 2) * P] = WkT[
                :, h1 * 64 : h1 * 64 + 64
            ]
            bqk[0:64, 2 * pr] = bq[h0 * 64 : h0 * 64 + 64]
            bqk[64:128, 2 * pr] = bq[h1 * 64 : h1 * 64 + 64]
            bqk[0:64, 2 * pr + 1] = bk[h0 * 64 : h0 * 64 + 64]
            bqk[64:128, 2 * pr + 1] = bk[h1 * 64 : h1 * 64 + 64]

        wv = np.ascontiguousarray(WvT[:, heads[0] * 64 : (heads[-1] + 1) * 64])
        bvr_v = bv[heads[0] * 64 : (heads[-1] + 1) * 64]

        xT = np.ascontiguousarray(x[b].T)  # [D, S]
        if USE_FP8_PROJ:
            xq = dr_pack(xT).astype(f8e4)
            wqk_in = dr_pack(wqk * W8).astype(f8e4)
            wv_in = dr_pack(wv * W8).astype(f8e4)
            bqk_in = bqk * W8
            bvr_in = np.ascontiguousarray(
                np.broadcast_to(bvr_v[None, :] * W8, (P, 256))
            ).astype(np.float32)
        else:
            xq = xT.reshape(8, P, S).transpose(1, 0, 2).astype(bf)
            xq = np.ascontiguousarray(xq)
            wqk_in = np.ascontiguousarray(
                wqk.reshape(8, P, 512).transpose(1, 0, 2)
            ).astype(bf)
            wv_in = np.ascontiguousarray(
                wv.reshape(8, P, 256).transpose(1, 0, 2)
            ).astype(bf)
            bqk_in = bqk
            bvr_in = np.ascontiguousarray(
                np.broadcast_to(bvr_v[None, :], (P, 256))
            ).astype(np.float32)

        m = maskf[b]  # [S]
        ct = (C_prior[b].T * m[:, None]).astype(bf)  # [S(k), S(q)]
        # exp bias per (k-partition, strip): round-to-nearest offset + mask
        mkb = np.full((P, NK), ROUND_BIAS, np.float32)
        mkb += np.where(m.reshape(NK, P).T > 0, 0.0, -1e9).astype(np.float32)
        vs = W8 if USE_FP8_PROJ else 1.0
        onesb = np.full((P, 64), vs, bf)

        in_maps.append(
            {
                "xq": xq,
                "wqk": wqk_in,
                "wv": wv_in,
                "bqk": bqk_in,
                "bvr": bvr_in,
                "ct": ct,
                "mkb": mkb,
                "onesb": onesb,
            }
        )

    nc = _get_program()
    trace = bool(int(os.environ.get("BASS_KERNEL_TRACE", "0")))
    res = run_bass_kernel_spmd(nc, in_maps, list(range(NCORES)), trace=trace)
    if trace:
        print(f"HW exec time: {res.exec_time_ns} ns")
        _prog_cache["last_exec_time_ns"] = res.exec_time_ns
        _prog_cache["last_trace"] = res.instructions_and_trace

    out = np.empty((B, S, D), np.float32)
    for c in range(NCORES):
        b, hg = c // 4, c % 4
        co = res.results[c]["out"]  # [256, S]
        for i in range(HEADS_PER_CORE):
            h = 4 * hg + i
            out[b, :, h * 64 : (h + 1) * 64] = co[i * 64 : (i + 1) * 64, :].T
    return out


# revision 20
# speedup vs baseline: 1.1308x; 1.0011x over previous
"""Trainium2 Bass kernel for BertSelfAttention with C_prior multiply.

Reference (per batch b):
  q/k/v = x @ W{q,k,v}.T + b{q,k,v}            -> [S, D], H=16 heads of W=64
  scores = q k^T / sqrt(W); mask; softmax over k
  attn = softmax(scores) * C_prior[b]
  out = attn @ v                               -> [B, S, D]

Shapes: B=2, S=2048, D=1024, H=16, W=64.
Sharding: 8 cores; core c owns batch b=c//4 and 4 heads (hg=c%4).

Measured HW model driving the design: PE matmul time ~ max(PSUM f32
writes / 265G/s, rhs cols / 3.9G/s); Act = 128 lanes @1.2GHz; DVE gets
2x on 16-bit SBUF ops; Pool engine (nc.gpsimd) is a second 1.2GHz
vector engine.

Design vs the naive version:
  - Projections in fp8e4m3 DoubleRow: contraction 256 per step -> 4
    PSUM accumulation passes instead of 8 (proj PE time halves).
    Weights are pre-scaled x8 on host (so they sit in e4m3 normal
    range); compensated via the exp scale (q,k) and the broadcast
    constant (v).
  - scores^T [k,q] per (strip, head) land in a [128,1024] PSUM tile
    (2 banks); ONE exp per (strip, head) -> halves Act instruction count.
  - e is stored fp16. The softmax denominator is computed by a
    DoubleRow fp8e5m2 matmul whose rhs is the strided high-byte view
    of the fp16 e tile (fp16 high byte == e5m2 truncation). exp is
    biased by ln(1.0625) (half an m2 ulp) so truncation becomes
    round-to-nearest; the 1.0625 cancels between numerator and
    denominator. Thin [1,512] outputs: denominator streams 2 strips
    per call and writes almost nothing.
  - 1/denom via thin reciprocal (DVE) then broadcast up to 128
    partitions with a tiny f32r matmul whose host constant also holds
    the 1/8 fp8-V compensation.
  - mask folded into the per-partition exp bias; attn*C multiplies
    split between DVE and Pool; projection bias-adds (PSUM->SBUF
    moves) on Pool.
"""

import math
import os

import numpy as np
import ml_dtypes

B, S, D, H, W = 2, 2048, 1024, 16, 64
NCORES = 8
HEADS_PER_CORE = 4
P = 128
QH = S // 2  # q processed in two halves of 1024
NK = S // P  # 16 k-strips
BOFF = 8  # pass B trails pass A by this many strips

USE_FP8_PROJ = False
ROUND_BIAS = 0.0  # no pre-scale: truncation bias is scale-invariant
E5_TRUNC_RATIO = 0.91578  # E[e5m2_trunc(x)/x] for exp(N(0,1)) mantissas
W8 = 8.0  # host pre-scale on all projection weights (fp8 range)

_prog_cache = {}


def _build_program():
    import concourse.mybir as mybir
    import concourse.tile as tile
    from concourse import bacc

    dt = mybir.dt
    f32, bf16, f16 = dt.float32, dt.bfloat16, dt.float16
    f8e4, f8e5, f32r = dt.float8e4, dt.float8e5, dt.float32r
    Alu = mybir.AluOpType
    Act = mybir.ActivationFunctionType
    DR = mybir.MatmulPerfMode.DoubleRow

    nc = bacc.Bacc("TRN2", target_bir_lowering=False)

    if USE_FP8_PROJ:
        # x/W laid out for DoubleRow: contraction d = t*256 + i*128 + p
        xq_d = nc.declare_dram_parameter("xq", [P, 4, 2, S], f8e4, isOutput=False)
        wqk_d = nc.declare_dram_parameter("wqk", [P, 4, 2, 512], f8e4, isOutput=False)
        wv_d = nc.declare_dram_parameter("wv", [P, 4, 2, 256], f8e4, isOutput=False)
    else:
        xq_d = nc.declare_dram_parameter("xq", [P, 8, S], bf16, isOutput=False)
        wqk_d = nc.declare_dram_parameter("wqk", [P, 8, 512], bf16, isOutput=False)
        wv_d = nc.declare_dram_parameter("wv", [P, 8, 256], bf16, isOutput=False)
    bqk_d = nc.declare_dram_parameter("bqk", [P, 4], f32, isOutput=False)
    bvr_d = nc.declare_dram_parameter("bvr", [P, 256], f32, isOutput=False)
    ct_d = nc.declare_dram_parameter("ct", [S, S], bf16, isOutput=False)
    mkb_d = nc.declare_dram_parameter("mkb", [P, NK], f32, isOutput=False)
    onesb_d = nc.declare_dram_parameter("onesb", [P, 64], bf16, isOutput=False)
    out_d = nc.declare_dram_parameter("out", [256, S], f32, isOutput=True)

    EXPSCALE = 0.125 / (W8 * W8) if USE_FP8_PROJ else 0.125

    with tile.TileContext(nc) as tc:
        with tc.tile_pool(name="persist", bufs=1) as persist:
            qk_all = persist.tile([P, 4, S], bf16)
            v_sb = persist.tile([P, NK, 256], bf16)
            bqk_sb = persist.tile([P, 4], f32)
            bvr_sb = persist.tile([P, 256], f32)
            mkb_sb = persist.tile([P, NK], f32)
            onesb_sb = persist.tile([P, 64], bf16)
            if USE_FP8_PROJ:
                xq_sb = persist.tile([P, 4, 2, S], f8e4)
                wqk_sb = persist.tile([P, 4, 2, 512], f8e4)
                wv_sb = persist.tile([P, 4, 2, 256], f8e4)
            else:
                xq_sb = persist.tile([P, 8, S], bf16)
                wqk_sb = persist.tile([P, 8, 512], bf16)
                wv_sb = persist.tile([P, 8, 256], bf16)
            nc.sync.dma_start(out=bqk_sb[:], in_=bqk_d[:])
            nc.sync.dma_start(out=bvr_sb[:], in_=bvr_d[:])
            nc.sync.dma_start(out=mkb_sb[:], in_=mkb_d[:])
            nc.sync.dma_start(out=onesb_sb[:], in_=onesb_d[:])
            for t in range(8):
                nc.sync.dma_start(out=wqk_sb[:, t], in_=wqk_d[:, t])
                nc.sync.dma_start(
                    out=xq_sb[:, t, 0:512], in_=xq_d[:, t, 0:512]
                )
            for qb in range(1, 4):
                for t in range(8):
                    nc.sync.dma_start(
                        out=xq_sb[:, t, qb * 512 : (qb + 1) * 512],
                        in_=xq_d[:, t, qb * 512 : (qb + 1) * 512],
                    )
            for t in range(8):
                nc.sync.dma_start(out=wv_sb[:, t], in_=wv_d[:, t])

            with tc.tile_pool(name="estr", bufs=16) as ep, tc.tile_pool(
                name="astr", bufs=12
            ) as app, tc.tile_pool(name="ctp", bufs=8) as ctp, tc.tile_pool(
                name="small", bufs=1
            ) as smallp, tc.tile_pool(
                name="mm1ps", bufs=4, space="PSUM"
            ) as mm1p, tc.tile_pool(
                name="ops", bufs=1, space="PSUM"
            ) as pop, tc.tile_pool(
                name="rsps", bufs=1, space="PSUM"
            ) as prsp:

                def _proj_step(item, ps, t):
                    if item[0] == "qk":
                        col, qb = item[1], item[2]
                        nc.tensor.matmul(
                            ps[:],
                            lhsT=wqk_sb[:, t, col * P : (col + 1) * P],
                            rhs=xq_sb[:, t, qb * 512 : (qb + 1) * 512],
                            start=(t == 0),
                            stop=(t == 7),
                        )
                    else:
                        kt = item[1]
                        nc.tensor.matmul(
                            ps[:, 0:256],
                            lhsT=xq_sb[:, t, kt * P : (kt + 1) * P],
                            rhs=wv_sb[:, t],
                            start=(t == 0),
                            stop=(t == 7),
                        )

                def _proj_fin(item, ps):
                    if item[0] == "qk":
                        col, qb = item[1], item[2]
                        nc.vector.tensor_scalar_add(
                            out=qk_all[:, col, qb * 512 : (qb + 1) * 512],
                            in0=ps[:],
                            scalar1=bqk_sb[:, col : col + 1],
                        )
                    else:
                        kt = item[1]
                        nc.vector.tensor_tensor(
                            v_sb[:, kt, :], ps[:, 0:256], bvr_sb[:], Alu.add
                        )

                def proj_emit(items):
                    # interleave steps of up to 2 calls: consecutive matmuls
                    # hit different PSUM regions, so accumulation RMW drains
                    # overlap instead of serializing.
                    tiles = [
                        mm1p.tile([P, 512], f32, tag="scT", name="pj")
                        for _ in items
                    ]
                    for t in range(8):
                        for it, ps in zip(items, tiles):
                            _proj_step(it, ps, t)
                    for it, ps in zip(items, tiles):
                        _proj_fin(it, ps)

                def passA_strip(qh, pr, ks, eA, eB):
                    par = ks % 2
                    for q2 in range(2):
                        qs = slice(qh * QH + q2 * 512, qh * QH + (q2 + 1) * 512)
                        os_ = slice(q2 * 512, (q2 + 1) * 512)
                        psA = mm1p.tile([P, 512], f32, tag="scT", name="psA")
                        psB = mm1p.tile([P, 512], f32, tag="scT", name="psB")
                        nc.tensor.matmul(
                            psA[:],
                            lhsT=qk_all[0:64, 2 * pr + 1, ks * P : (ks + 1) * P],
                            rhs=qk_all[0:64, 2 * pr, qs],
                            tile_position=(0, 0),
                            start=True,
                            stop=True,
                        )
                        nc.tensor.matmul(
                            psB[:],
                            lhsT=qk_all[64:128, 2 * pr + 1, ks * P : (ks + 1) * P],
                            rhs=qk_all[64:128, 2 * pr, qs],
                            tile_position=(64, 0),
                            start=True,
                            stop=True,
                        )
                        nc.scalar.activation(
                            eA[:, par, os_],
                            psA[:],
                            Act.Exp,
                            scale=EXPSCALE,
                            bias=mkb_sb[:, ks : ks + 1],
                        )
                        nc.scalar.activation(
                            eB[:, par, os_],
                            psB[:],
                            Act.Exp,
                            scale=EXPSCALE,
                            bias=mkb_sb[:, ks : ks + 1],
                        )

                def prepB_strip(qh, pr, ks, eA, eB):
                    par = ks % 2
                    ct = ctp.tile([P, QH], bf16, tag="ct")
                    nc.sync.dma_start(
                        out=ct[:],
                        in_=ct_d[ks * P : (ks + 1) * P, qh * QH : (qh + 1) * QH],
                    )
                    aA = app.tile([P, QH], bf16, tag="a")
                    aB = app.tile([P, QH], bf16, tag="a")
                    nc.vector.tensor_tensor(aA[:], eA[:, par, :], ct[:], Alu.mult)
                    nc.vector.tensor_tensor(aB[:], eB[:, par, :], ct[:], Alu.mult)
                    return aA, aB

                def passB_strip(qh, pr, ks, eA, eB, aA, aB, po, prs):
                    h0, h1 = 2 * pr, 2 * pr + 1
                    par = ks % 2
                    st, sp = (ks == 0), (ks == NK - 1)
                    for q2 in range(2):
                        os_ = slice(q2 * 512, (q2 + 1) * 512)
                        nc.tensor.matmul(
                            po[0:64, os_],
                            lhsT=v_sb[:, ks, h0 * 64 : (h0 + 1) * 64],
                            rhs=aA[:, os_],
                            tile_position=(0, 0),
                            start=st,
                            stop=sp,
                        )
                        nc.tensor.matmul(
                            po[64:128, os_],
                            lhsT=v_sb[:, ks, h1 * 64 : (h1 + 1) * 64],
                            rhs=aB[:, os_],
                            tile_position=(0, 64),
                            start=st,
                            stop=sp,
                        )
                    # denominators: regular f16 ones matmuls per strip
                    for q2 in range(2):
                        os_ = slice(q2 * 512, (q2 + 1) * 512)
                        nc.tensor.matmul(
                            prs[0:64, os_],
                            lhsT=onesb_sb[:],
                            rhs=eA[:, par, os_],
                            tile_position=(0, 0),
                            start=st,
                            stop=sp,
                        )
                        nc.tensor.matmul(
                            prs[64:128, os_],
                            lhsT=onesb_sb[:],
                            rhs=eB[:, par, os_],
                            tile_position=(0, 64),
                            start=st,
                            stop=sp,
                        )

                def finishB(qh, pr, po, prs):
                    rcs = smallp.tile([P, QH], f32, tag="rcs")
                    scr = smallp.tile([P, QH], f32, tag="scr")
                    nc.vector.reciprocal_approx_accurate(rcs[:], prs[:], scr[:])
                    ob = smallp.tile([P, QH], f32, tag="ob")
                    for q2 in range(2):
                        os_ = slice(q2 * 512, (q2 + 1) * 512)
                        nc.vector.tensor_tensor(
                            ob[:, os_], po[:, os_], rcs[:, os_], Alu.mult
                        )
                        nc.sync.dma_start(
                            out=out_d[
                                pr * P : (pr + 1) * P,
                                qh * QH + q2 * 512 : qh * QH + (q2 + 1) * 512,
                            ],
                            in_=ob[:, os_],
                        )

                phases = [(qh, pr) for qh in range(2) for pr in range(2)]

                # projection schedule: (col, qb) for qk / ('v', kt) for v,
                # ordered by need-by iteration; drained ~1/iteration.
                projq = (
                    [("qk", 1, 1), ("qk", 1, 2), ("v", 0), ("v", 1)]
                    + [("qk", 1, 3), ("v", 2), ("v", 3), ("v", 4), ("v", 5)]
                    + [("qk", 2, 0), ("qk", 2, 1), ("qk", 3, 0)]
                    + [("v", 6), ("v", 7), ("v", 8), ("v", 9), ("qk", 3, 1)]
                    + [("v", 10), ("v", 11), ("v", 12), ("qk", 3, 2), ("v", 13)]
                    + [("v", 14), ("v", 15), ("qk", 3, 3)]
                    + [("qk", 0, 2), ("qk", 0, 3), ("qk", 2, 2), ("qk", 2, 3)]
                )
                deadlines = {
                    ("qk", 1, 1): 4, ("qk", 1, 2): 8, ("qk", 1, 3): 12,
                    ("qk", 2, 0): 16, ("qk", 2, 1): 16, ("qk", 3, 0): 16,
                    ("qk", 3, 1): 20, ("qk", 3, 2): 24, ("qk", 3, 3): 28,
                    ("qk", 0, 2): 32, ("qk", 0, 3): 32,
                    ("qk", 2, 2): 48, ("qk", 2, 3): 48,
                }
                for kt in range(NK):
                    deadlines[("v", kt)] = kt + BOFF



                # prologue: Q(pair0, qh0) + first K(pair0) block
                proj_emit([("qk", 0, 0), ("qk", 1, 0)])
                proj_emit([("qk", 0, 1)])

                es_all = {}
                bstate = {}
                prepped = {}
                pptr = [0]
                PREP_AHEAD = 4
                NITER = 4 * NK + BOFF
                for g in range(NITER):
                    ph, ks = g // NK, g % NK
                    if g < 4 * NK:
                        qh, pr = phases[ph]
                        if ks % 2 == 0:
                            eA = ep.tile([P, 2, QH], bf16, tag="e", name="eA")
                            eB = ep.tile([P, 2, QH], bf16, tag="e", name="eB")
                            es_all.setdefault(ph, []).append((eA, eB))
                        eA, eB = es_all[ph][ks // 2]
                        passA_strip(qh, pr, ks, eA, eB)
                    # drain projection queue (deadline-forced, else 1/iter)
                    batch = []
                    while pptr[0] < len(projq) and (
                        deadlines[projq[pptr[0]]] <= g + 2 or not batch
                    ):
                        batch.append(projq[pptr[0]])
                        pptr[0] += 1
                        if len(batch) >= 2 and not (
                            pptr[0] < len(projq)
                            and deadlines[projq[pptr[0]]] <= g + 2
                        ):
                            break
                    for i in range(0, len(batch), 2):
                        proj_emit(batch[i : i + 2])
                    pg = g - (BOFF - PREP_AHEAD)
                    if 0 <= pg < 4 * NK:
                        pph, pks = pg // NK, pg % NK
                        pqh, ppr = phases[pph]
                        peA, peB = es_all[pph][pks // 2]
                        prepped[pg] = prepB_strip(pqh, ppr, pks, peA, peB)
                    bg = g - BOFF
                    if bg >= 0:
                        bph, bks = bg // NK, bg % NK
                        bqh, bpr = phases[bph]
                        if bks == 0:
                            b_po = pop.tile([P, QH], f32, tag="po", name="po")
                            b_prs = prsp.tile([P, QH], f32, tag="prs", name="prs")
                            bstate[bph] = (b_po, b_prs)
                        b_po, b_prs = bstate[bph]
                        eA, eB = es_all[bph][bks // 2]
                        aA, aB = prepped.pop(bg)
                        passB_strip(bqh, bpr, bks, eA, eB, aA, aB, b_po, b_prs)
                        if bks == NK - 1:
                            finishB(bqh, bpr, b_po, b_prs)
                            del bstate[bph]
                            del es_all[bph]

    nc.finalize()
    return nc


def _get_program():
    if "nc" not in _prog_cache:
        _prog_cache["nc"] = _build_program()
    return _prog_cache["nc"]


def kernel(x, attention_mask, C_prior, Wq, bq, Wk, bk, Wv, bv):
    from concourse.bass_utils import run_bass_kernel_spmd

    x = np.asarray(x, dtype=np.float32)
    attention_mask = np.asarray(attention_mask)
    C_prior = np.asarray(C_prior, dtype=np.float32)
    Wq = np.asarray(Wq, dtype=np.float32)
    Wk = np.asarray(Wk, dtype=np.float32)
    Wv = np.asarray(Wv, dtype=np.float32)
    bq = np.asarray(bq, dtype=np.float32)
    bk = np.asarray(bk, dtype=np.float32)
    bv = np.asarray(bv, dtype=np.float32)
    bf = ml_dtypes.bfloat16
    f8e4 = ml_dtypes.float8_e4m3fn
    f8e5 = ml_dtypes.float8_e5m2

    WqT, WkT, WvT = Wq.T, Wk.T, Wv.T  # [in D, out D]
    maskf = attention_mask.astype(np.float32)  # [B, S]

    def dr_pack(a):
        # [D, M] -> [128, 4, 2, M] with d = t*256 + i*128 + p
        Dd, M = a.shape
        return np.ascontiguousarray(
            a.reshape(4, 2, P, M).transpose(2, 0, 1, 3)
        )

    in_maps = []
    for c in range(NCORES):
        b, hg = c // 4, c % 4
        heads = [4 * hg + i for i in range(HEADS_PER_CORE)]

        wqk = np.empty((D, 512), np.float32)
        bqk = np.zeros((P, 4), np.float32)
        for pr in range(2):
            h0, h1 = heads[2 * pr], heads[2 * pr + 1]
            wqk[:, (2 * pr) * P : (2 * pr) * P + 64] = WqT[:, h0 * 64 : h0 * 64 + 64]
            wqk[:, (2 * pr) * P + 64 : (2 * pr + 1) * P] = WqT[
                :, h1 * 64 : h1 * 64 + 64
            ]
            wqk[:, (2 * pr + 1) * P : (2 * pr + 1) * P + 64] = WkT[
                :, h0 * 64 : h0 * 64 + 64
            ]
            wqk[:, (2 * pr + 1) * P + 64 : (2 * pr + 2) * P] = WkT[
                :, h1 * 64 : h1 * 64 + 64
            ]
            bqk[0:64, 2 * pr] = bq[h0 * 64 : h0 * 64 + 64]
            bqk[64:128, 2 * pr] = bq[h1 * 64 : h1 * 64 + 64]
            bqk[0:64, 2 * pr + 1] = bk[h0 * 64 : h0 * 64 + 64]
            bqk[64:128, 2 * pr + 1] = bk[h1 * 64 : h1 * 64 + 64]

        wv = np.ascontiguousarray(WvT[:, heads[0] * 64 : (heads[-1] + 1) * 64])
        bvr_v = bv[heads[0] * 64 : (heads[-1] + 1) * 64]

        xT = np.ascontiguousarray(x[b].T)  # [D, S]
        if USE_FP8_PROJ:
            xq = dr_pack(xT).astype(f8e4)
            wqk_in = dr_pack(wqk * W8).astype(f8e4)
            wv_in = dr_pack(wv * W8).astype(f8e4)
            bqk_in = bqk * W8
            bvr_in = np.ascontiguousarray(
                np.broadcast_to(bvr_v[None, :] * W8, (P, 256))
            ).astype(np.float32)
        else:
            xq = xT.reshape(8, P, S).transpose(1, 0, 2).astype(bf)
            xq = np.ascontiguousarray(xq)
            wqk_in = np.ascontiguousarray(
                wqk.reshape(8, P, 512).transpose(1, 0, 2)
            ).astype(bf)
            wv_in = np.ascontiguousarray(
                wv.reshape(8, P, 256).transpose(1, 0, 2)
            ).astype(bf)
            bqk_in = bqk
            bvr_in = np.ascontiguousarray(
                np.broadcast_to(bvr_v[None, :], (P, 256))
            ).astype(np.float32)

        m = maskf[b]  # [S]
        ct = (C_prior[b].T * m[:, None]).astype(bf)  # [S(k), S(q)]
        # exp bias per (k-partition, strip): round-to-nearest offset + mask
        mkb = np.full((P, NK), ROUND_BIAS, np.float32)
        mkb += np.where(m.reshape(NK, P).T > 0, 0.0, -1e9).astype(np.float32)
        vs = W8 if USE_FP8_PROJ else 1.0
        onesb = np.full((P, 64), vs, bf)

        in_maps.append(
            {
                "xq": xq,
                "wqk": wqk_in,
                "wv": wv_in,
                "bqk": bqk_in,
                "bvr": bvr_in,
                "ct": ct,
                "mkb": mkb,
                "onesb": onesb,
            }
        )

    nc = _get_program()
    trace = bool(int(os.environ.get("BASS_KERNEL_TRACE", "0")))
    res = run_bass_kernel_spmd(nc, in_maps, list(range(NCORES)), trace=trace)
    if trace:
        print(f"HW exec time: {res.exec_time_ns} ns")
        _prog_cache["last_exec_time_ns"] = res.exec_time_ns
        _prog_cache["last_trace"] = res.instructions_and_trace

    out = np.empty((B, S, D), np.float32)
    for c in range(NCORES):
        b, hg = c // 4, c % 4
        co = res.results[c]["out"]  # [256, S]
        for i in range(HEADS_PER_CORE):
            h = 4 * hg + i
            out[b, :, h * 64 : (h + 1) * 64] = co[i * 64 : (i + 1) * 64, :].T
    return out
